# revision 50
# baseline (speedup 1.0000x reference)
"""Trainium2 Bass kernel for nn_CFHoTWrapper (sparse attention with adapter gate).

Sharding: tensor-parallel over attention heads across 8 NeuronCores.
Each core computes 4 query heads + its 1 KV head end-to-end (QKV proj,
RoPE, scores, softmax, AV, partial O-projection); the tiny adapter gate
is replicated on every core. Per-core partial outputs (bf16) are summed
on the host.

Softmax is computed without max-subtraction (scores are O(5) for these
shapes so exp() is safe in fp32), and the per-key gate bias is folded in
multiplicatively: exp(s + m + g[k]) = exp(s) * exp(m) * w[k] with
w = exp(gate_scale * gate).  w scales the V rows, and an extra all-w
column appended to V yields the softmax denominator from the same
matmul that computes the numerator.

AV is computed TRANSPOSED: stationary = augmented V block [keys, 65],
moving = exp'd score tile [keys, q-chunk].  PSUM accumulates over key
blocks with partial-width matmuls (per-element has_written handles the
causal staircase).  This kills the per-(i,j) LDWEIGHTS reloads of the
score tiles (the old AV was weight-load-bound) AND produces attn^T
directly in the [head_dim, q] layout the O-projection needs as its
stationary operand - no PE transposes, no aT copies.  The softmax
denominator lands in psum row 64; its reciprocal row is broadcast to
64 partitions with a K=1 fp32r ones-matmul and folded into the
psum->SBUF cast.

RoPE's rotate-half is a fixed row permutation, done as a PE matmul with
a permutation matrix (sign folded into the sin table) instead of
SBUF->SBUF DMA.  The adapter projection is column-tiled: even k-slices
land in psum partitions 0-63 (tile_position (0,0)), odd slices in
64-127 ((0,64)), so pairs of slices stream concurrently through the PE
and the two half-sums are merged by one DVE scalar_tensor_tensor.

DMA order keeps both HWDGE queues streaming hT from instruction 0
(small early weights first, all later-phase weights behind the hT
stream) so the PE's k-outer passes never starve and TRN2's HAM clock
gate stays at full rate.
"""

import math
import os
from contextlib import ExitStack

import numpy as np
import ml_dtypes

import concourse.bass as bass
import concourse.tile as tile
from concourse import mybir
from concourse.masks import make_identity
from concourse.bass_utils import run_bass_kernel_spmd

BF16 = ml_dtypes.bfloat16
F32 = np.float32

S = 2048
D = 2048
HD = 64
NH = 32
NKV = 8
NCORES = 8
HLOC = NH // NCORES          # 4 query heads per core
P = 128
NT = S // P                  # 16 sequence tiles of 128
NCH = 4                      # 4 sequence chunks of 512
CH = 512
ALPHA = 0.995
MASK_NEG_THRESH = -80.0      # exp() underflows to 0 below this

LAST_RESULT = None           # BassKernelResults of the last run (for test.py)


def _analyze_mask(maskT):
    """Classify [keys=128 x q=128] blocks of maskT and dedup non-trivial
    multiplicative (exp) mask patterns. maskT is [S, S] (keys, q)."""
    mb = [[None] * NT for _ in range(NT)]
    patterns = []
    pat_index = {}
    for j in range(NT):
        for i in range(NT):
            blk = maskT[j * P:(j + 1) * P, i * P:(i + 1) * P]
            if (blk < MASK_NEG_THRESH).all():
                mb[j][i] = 'skip'
            elif (blk == 0.0).all():
                mb[j][i] = 'plain'
            else:
                pat = np.exp(np.minimum(blk, 80.0)).astype(BF16)
                key = pat.tobytes()
                if key not in pat_index:
                    pat_index[key] = len(patterns)
                    patterns.append(pat)
                mb[j][i] = pat_index[key]
    av_incl = [[j for j in range(NT) if mb[j][i] != 'skip'] for i in range(NT)]
    return mb, patterns, av_incl


def _split_sync_waits(nc):
    """This walrus build supports only ONE embedded sync wait per
    instruction; hoist extra waits onto preceding sequencer NoOps."""
    for f in nc.m.functions:
        for bb in f.blocks:
            insts = bb.instructions
            idx = 0
            while idx < len(insts):
                inst = insts[idx]
                si = inst.sync_info
                if si is not None and si.on_wait and len(si.on_wait) > 1:
                    waits = list(si.on_wait)
                    for w in waits[:-1]:
                        nop = mybir.InstNoOp(
                            name=nc.get_next_instruction_name(),
                            engine=inst.engine,
                            sync_info=mybir.SyncInfo(on_wait=[w], on_update=[]),
                            bass_nofuse=True,
                        )
                        nc.register_instruction(nop)
                        insts.insert(idx, nop)
                        idx += 1
                    inst.sync_info = mybir.SyncInfo(
                        on_wait=[waits[-1]], on_update=list(si.on_update))
                idx += 1


def _build_program(mb, n_pat, av_incl, field_scale, b2_scaled, gate_scale):
    nc = bass.Bass()
    dt = mybir.dt

    hT_d = nc.declare_dram_parameter("hT", [D, S], dt.bfloat16, isOutput=False)
    wq_d = nc.declare_dram_parameter("wq", [P, NT * HLOC * HD], dt.bfloat16, isOutput=False)
    wkv_d = nc.declare_dram_parameter("wkv", [P, NT * 2 * HD], dt.bfloat16, isOutput=False)
    wo_d = nc.declare_dram_parameter("wo", [P, 2 * D], dt.bfloat16, isOutput=False)
    w1a_d = nc.declare_dram_parameter("w1a", [P, NT * 64], dt.bfloat16, isOutput=False)
    w2_d = nc.declare_dram_parameter("w2", [64, 1], dt.bfloat16, isOutput=False)
    b1_d = nc.declare_dram_parameter("b1", [64, 1], dt.float32, isOutput=False)
    cos2q_d = nc.declare_dram_parameter("cos2q", [P, S], dt.bfloat16, isOutput=False)
    sin2q_d = nc.declare_dram_parameter("sin2q", [P, S], dt.bfloat16, isOutput=False)
    cosk_d = nc.declare_dram_parameter("cosk", [HD, S], dt.bfloat16, isOutput=False)
    sink_d = nc.declare_dram_parameter("sink", [HD, S], dt.bfloat16, isOutput=False)
    pq_d = nc.declare_dram_parameter("pq", [P, P], dt.bfloat16, isOutput=False)
    sel2_d = nc.declare_dram_parameter("sel2", [HD + 1, P], dt.bfloat16, isOutput=False)
    if n_pat:
        pm_d = nc.declare_dram_parameter("pmask", [n_pat, P, P], dt.bfloat16, isOutput=False)
    out_d = nc.declare_dram_parameter("out", [S, D], dt.bfloat16, isOutput=True)

    with tile.TileContext(nc) as tc, ExitStack() as ctx:
        pers = ctx.enter_context(tc.tile_pool(name="pers", bufs=1))
        psp = ctx.enter_context(tc.tile_pool(name="psum", bufs=4, space="PSUM"))

        # persistent (phase-C-lifetime) tiles; DMAs are emitted inside the
        # phb block below so small early-needed weights go first on the queue
        w2 = pers.tile([64, 1], dt.bfloat16)
        b1 = pers.tile([64, 1], dt.float32)
        pqm = pers.tile([P, P], dt.bfloat16)
        hT = pers.tile([P, NT, S], dt.bfloat16)
        wq = pers.tile([P, NT, HLOC * HD], dt.bfloat16)
        if n_pat:
            pmask = pers.tile([P, n_pat, P], dt.bfloat16)
        wo = pers.tile([P, 2, D], dt.bfloat16)
        ident = pers.tile([P, P], dt.bfloat16)
        make_identity(nc, ident)
        # selector for the denominator broadcast: contraction row 0 -> ones
        # on out rows 0-63 (head A), row 64 -> ones on 64-127 (head B); all
        # other rows are zero, so one K=65 matmul fans both rows out
        sel2 = pers.tile([HD + 1, P], dt.bfloat16)


        qt_pair = [pers.tile([P, S], dt.bfloat16, tag=f"qp{t}", name=f"qp{t}")
                   for t in range(HLOC // 2)]
        kt = pers.tile([HD, S], dt.bfloat16)
        ktp = pers.tile([P, S], dt.bfloat16)      # kt duplicated at base 64
        vaug = pers.tile([P, NT, HD + 1], dt.bfloat16)
        wcol = pers.tile([P, NT], dt.float32)
        cos2q = pers.tile([P, S], dt.bfloat16)
        sin2q = pers.tile([P, S], dt.bfloat16)

        phbw = ctx.enter_context(tc.tile_pool(name="phbw", bufs=2))
        with tc.tile_pool(name="phb", bufs=1) as phb:
            # ---- load order: tiny early-phase weights lead, then BOTH
            # HWDGE queues stream hT back-to-back (even k on SP, odd on
            # ACT); all later-phase weights queue BEHIND the hT stream so
            # the k-outer passes trailing the stream never starve ----
            w1a = phb.tile([P, NT, 64], dt.bfloat16)
            nc.scalar.dma_start(out=w1a, in_=w1a_d[:, :])
            wkv = phb.tile([P, NT, 2 * HD], dt.bfloat16)
            nc.scalar.dma_start(out=wkv, in_=wkv_d[:, :])
            nc.sync.dma_start(out=w2, in_=w2_d[:, :])
            nc.sync.dma_start(out=b1, in_=b1_d[:, :])
            nc.sync.dma_start(out=pqm, in_=pq_d[:, :])
            nc.sync.dma_start(out=sel2, in_=sel2_d[:, :])
            for k in range(NT):
                eng = nc.sync if k % 2 == 0 else nc.scalar
                eng.dma_start(out=hT[:, k, :], in_=hT_d[k * P:(k + 1) * P, :])
            cosk = phb.tile([HD, S], dt.bfloat16)
            nc.sync.dma_start(out=cosk, in_=cosk_d[:, :])
            sink = phb.tile([HD, S], dt.bfloat16)
            nc.sync.dma_start(out=sink, in_=sink_d[:, :])
            nc.scalar.dma_start(out=wq, in_=wq_d[:, :])
            nc.sync.dma_start(out=cos2q, in_=cos2q_d[:, :])
            nc.scalar.dma_start(out=sin2q, in_=sin2q_d[:, :])
            if n_pat:
                for m in range(n_pat):
                    nc.sync.dma_start(out=pmask[:, m, :], in_=pm_d[m, :, :])
            nc.sync.dma_start(out=wo, in_=wo_d[:, :])

            # --- pass 1a+1b interleaved per k-slice so consumption
            # tracks the hT DMA stream ---
            accA = [psp.tile([64, CH], dt.float32, tag="sc", bufs=4,
                             name=f"accA{c}") for c in range(NCH)]
            accKV = [psp.tile([P, CH], dt.float32, tag="sc", bufs=4,
                              name=f"accKV{c}") for c in range(NCH)]
            for k in range(NT):
                for c in range(NCH):
                    nc.tensor.matmul(accA[c], w1a[:, k, :],
                                     hT[:, k, c * CH:(c + 1) * CH],
                                     start=(k == 0), stop=(k == NT - 1))
                    nc.tensor.matmul(accKV[c], wkv[:, k, :],
                                     hT[:, k, c * CH:(c + 1) * CH],
                                     start=(k == 0), stop=(k == NT - 1))
            hmT = phb.tile([64, S], dt.bfloat16)
            kraw = phb.tile([HD, S], dt.bfloat16)
            vt = phb.tile([HD, S], dt.bfloat16)
            for c in range(NCH):
                csl = slice(c * CH, (c + 1) * CH)
                # exact gelu(x) = 0.5 * x * (1 + erf(x / sqrt(2))), x = ps + b1
                pre = phbw.tile([64, CH], dt.float32, tag="pre")
                nc.vector.tensor_scalar(pre, accA[c], b1, None, mybir.AluOpType.add)
                er = phbw.tile([64, CH], dt.float32, tag="er")
                nc.scalar.activation(er, pre, mybir.ActivationFunctionType.Erf,
                                     bias=0.0, scale=1.0 / math.sqrt(2.0))
                nc.vector.tensor_scalar(er, er, 0.5, 0.5,
                                        mybir.AluOpType.mult, mybir.AluOpType.add)
                nc.vector.tensor_mul(hmT[:, csl], pre, er)
                nc.vector.tensor_copy(kraw[:, csl], accKV[c][0:HD, :])
                nc.vector.tensor_copy(vt[:, csl], accKV[c][HD:P, :])

            # --- K RoPE: rotate-half via PE permutation matmul ---
            for c in range(NCH):
                csl = slice(c * CH, (c + 1) * CH)
                pmk = psp.tile([HD, CH], dt.float32, tag="od", bufs=2)
                nc.tensor.matmul(pmk, pqm[0:HD, 0:HD], kraw[:, csl],
                                 start=True, stop=True)
                t1k = phbw.tile([HD, CH], dt.bfloat16, tag="t1")
                nc.vector.tensor_mul(t1k, kraw[:, csl], cosk[:, csl])
                t2k = phbw.tile([HD, CH], dt.bfloat16, tag="t2")
                nc.vector.tensor_mul(t2k, pmk, sink[:, csl])
                nc.vector.tensor_add(kt[:, csl], t1k, t2k)
            nc.sync.dma_start(out=ktp[HD:P, :], in_=kt[:, :])

            # --- V tiles: PE transpose into unscaled vraw now (dep-free);
            # the wcol gate scaling runs later so the slow field chain
            # never stalls the in-order PE queue ---
            vraw = phbw.tile([P, NT, HD], dt.bfloat16, tag="vraw", bufs=1)
            for st in range(NT):
                pv = psp.tile([P, HD], dt.bfloat16, tag="od", bufs=2)
                nc.tensor.transpose(pv, vt[:, st * P:(st + 1) * P],
                                    ident[0:HD, 0:HD])
                nc.vector.tensor_copy(vraw[:, st, :], pv)

            # --- field row = field_scale * (hmidT^T @ W2 + b2); gate ---
            field = phb.tile([1, S], dt.float32)
            scratch = phb.tile([1, S], dt.float32)
            for c in range(NCH):
                ps = psp.tile([1, CH], dt.float32, tag="sc", bufs=4)
                nc.tensor.matmul(ps, w2, hmT[:, c * CH:(c + 1) * CH],
                                 start=True, stop=True)
                nc.vector.tensor_scalar(field[:, c * CH:(c + 1) * CH], ps,
                                        field_scale, b2_scaled,
                                        mybir.AluOpType.mult, mybir.AluOpType.add)
            ssum = phb.tile([1, 1], dt.float32)
            nc.vector.reduce_sum(ssum, field, axis=mybir.AxisListType.X)
            mean = phb.tile([1, 1], dt.float32)
            nc.vector.tensor_scalar_mul(mean, ssum, 1.0 / S)
            nc.vector.tensor_scalar(field, field, mean, None, mybir.AluOpType.subtract)
            nc.scalar.square(scratch, field)
            ss2 = phb.tile([1, 1], dt.float32)
            nc.vector.reduce_sum(ss2, scratch, axis=mybir.AxisListType.X)
            std = phb.tile([1, 1], dt.float32)
            nc.scalar.activation(std, ss2, mybir.ActivationFunctionType.Sqrt,
                                 bias=0.0, scale=1.0 / (S - 1))
            nc.vector.tensor_scalar_add(std, std, 1e-6)
            rstd = phb.tile([1, 1], dt.float32)
            nc.vector.reciprocal(rstd, std)
            gsr = phb.tile([1, 1], dt.float32)
            nc.vector.tensor_scalar_mul(gsr, rstd, gate_scale)
            # w row = exp(gate_scale * gate), into scratch
            nc.scalar.activation(scratch, field, mybir.ActivationFunctionType.Exp,
                                 bias=0.0, scale=gsr)
            # transpose the w row into per-partition columns [128, 16] via a
            # DRAM bounce (SBUF partitions are not element-addressable across
            # the partition stride, so an in-SBUF gather is illegal on HW).
            # Use the gpsimd SWDGE queue: independent of the two HWDGE rings
            # so the bounce never queues behind bulk weight traffic.
            wrow_dram = nc.dram_tensor("wrow_dram", [1, S], dt.float32)
            nc.gpsimd.dma_start(out=wrow_dram[:, :], in_=scratch)
            nc.gpsimd.dma_start(out=wcol,
                                in_=wrow_dram[0, :].rearrange("(j p) -> p j", p=P))


        # ------------- phase C setup: attention emit helpers -------------
        with tc.tile_pool(name="phc", bufs=2) as phc, \
             tc.tile_pool(name="phcs", bufs=4) as phcs, \
             tc.tile_pool(name="phd", bufs=2) as phd:

            def chunk_start(j, c):
                for ii in range(4 * c, 4 * c + 4):
                    if mb[j][ii] != 'skip':
                        return (ii % 4) * P
                return None

            pts_store = {}
            aT_store = {}

            def emit_scores(c, t):
                """Scores for head pair (2t, 2t+1): K=64 matmuls in PE
                row-groups 0 and 64 (A at base 0, B at base 64)."""
                ptsA, ptsB = {}, {}
                for j in range(NT):
                    s0 = chunk_start(j, c)
                    if s0 is None:
                        continue
                    jsl = slice(j * P, (j + 1) * P)
                    csl = slice(c * CH + s0, (c + 1) * CH)
                    psA = psp.tile([P, CH], dt.float32, tag="sc", bufs=4,
                                   name="ps_scA")
                    nc.tensor.matmul(psA[:, s0:CH], kt[:, jsl],
                                     qt_pair[t][0:HD, csl],
                                     start=True, stop=True, tile_position=(0, 0))
                    psB = psp.tile([P, CH], dt.float32, tag="sc", bufs=4,
                                   name="ps_scB")
                    nc.tensor.matmul(psB[:, s0:CH], ktp[HD:P, jsl],
                                     qt_pair[t][HD:P, csl],
                                     start=True, stop=True, tile_position=(64, 0))
                    for pts, ps, tagc in ((ptsA, psA, "pt"), (ptsB, psB, "pu")):
                        pt = phc.tile([P, CH], dt.bfloat16, tag=f"{tagc}{j}",
                                      name=f"{tagc}{j}")
                        nc.scalar.activation(pt[:, s0:CH], ps[:, s0:CH],
                                             mybir.ActivationFunctionType.Exp)
                        for ii in range(4 * c, 4 * c + 4):
                            kind = mb[j][ii]
                            if kind in ('skip', 'plain'):
                                continue
                            qq = slice((ii % 4) * P, (ii % 4 + 1) * P)
                            nc.vector.tensor_mul(pt[:, qq], pt[:, qq],
                                                 pmask[:, kind, :])
                        pts[j] = (pt, s0)
                pts_store[(c, 2 * t)] = ptsA
                pts_store[(c, 2 * t + 1)] = ptsB

            def emit_av(c, t):
                """Transposed AV for head pair t of chunk c: for each head,
                one psum [65, CH] accumulates stationary-V matmuls over key
                blocks (partial widths ride the per-element has_written
                bits).  Row 64 is the softmax denominator; its reciprocal
                row is PE-broadcast to 64 partitions and folded into the
                psum->SBUF cast, which writes attn^T for the head pair
                stacked [128, CH] - the O-projection stationary."""
                aTu = phcs.tile([P, CH], dt.bfloat16, tag="aTu", bufs=3,
                                name="aTu")
                den65 = phcs.tile([HD + 1, CH], dt.bfloat16, tag="den65",
                                  bufs=1, name="den65")
                # rows 1-63 feed the selector matmul with zero weights; they
                # must hold finite values (0 x NaN = NaN), so fill with 1.0
                nc.vector.memset(den65, 1.0)
                for hh in range(2):
                    h = 2 * t + hh
                    pts = pts_store.pop((c, h))
                    js = sorted(pts.keys())
                    pavT = psp.tile([HD + 1, CH], dt.float32, tag="av", bufs=2,
                                    name="ps_avT")
                    assert js and pts[js[0]][1] == 0, "first key block must span chunk"
                    for idx, j in enumerate(js):
                        pt, s0 = pts[j]
                        nc.tensor.matmul(pavT[:, s0:CH], vaug[:, j, :],
                                         pt[:, s0:CH],
                                         start=(idx == 0),
                                         stop=(idx == len(js) - 1))
                    # free the psum bank right away: numerators to SBUF
                    # (heads stacked), denominator row to partition hh*64
                    nc.vector.tensor_copy(aTu[hh * HD:(hh + 1) * HD, :],
                                          pavT[0:HD, :])
                    nc.vector.tensor_copy(den65[hh * HD:hh * HD + 1, :],
                                          pavT[HD:HD + 1, :])
                # one batched reciprocal covers both heads' denominator rows;
                # the broadcast + normalize run later (in this chunk's oproj
                # unit) so this slow DVE op is never on the PE critical path
                rcr65 = phcs.tile([HD + 1, CH], dt.bfloat16, tag="rcr65",
                                  bufs=3, name="rcr65")
                with nc.allow_low_precision(
                        reason="bf16 softmax-denominator reciprocal: 0.4% "
                               "per-query scale, inside the error budget"):
                    nc.vector.reciprocal(rcr65, den65)
                aT_store[(c, t)] = (aTu, rcr65)

            def emit_oproj(c, act_dc=0):
                # deferred normalize: the reciprocals were computed an entire
                # schedule unit ago, so the selector matmuls fire without
                # stalling the PE; then attn^T = unnormalized x broadcast rcp
                aTs = []
                parts = [aT_store.pop((c, t)) for t in range(HLOC // 2)]
                rcbs_t = []
                for t in range(HLOC // 2):
                    rcb = psp.tile([P, CH], dt.float32, tag="od", bufs=2,
                                   name="ps_rcb")
                    nc.tensor.matmul(rcb, sel2, parts[t][1],
                                     start=True, stop=True)
                    rcbs_t.append(rcb)
                for t in range(HLOC // 2):
                    rcbs = phcs.tile([P, CH], dt.bfloat16, tag="rcbs", bufs=2,
                                     name="rcbs")
                    nc.vector.tensor_copy(rcbs, rcbs_t[t])
                    aTt = phcs.tile([P, CH], dt.bfloat16, tag=f"aT{t}",
                                    bufs=1, name=f"aT{t}")
                    nc.vector.tensor_mul(aTt, parts[t][0], rcbs)
                    aTs.append(aTt)
                for i in range(4 * c, 4 * c + 4):
                    qsl = slice((i % 4) * P, (i % 4 + 1) * P)
                    ot = phd.tile([P, D], dt.bfloat16, tag="outsb", name="ot")
                    for dc in range(NCH):
                        po = psp.tile([P, CH], dt.float32, tag="od", bufs=2,
                                      name="ps_o")
                        dsl = slice(dc * CH, (dc + 1) * CH)
                        nc.tensor.matmul(po, aTs[0][:, qsl], wo[:, 0, dsl],
                                         start=True, stop=False)
                        nc.tensor.matmul(po, aTs[1][:, qsl], wo[:, 1, dsl],
                                         start=False, stop=True)
                        if dc < act_dc:
                            nc.scalar.copy(ot[:, dsl], po)
                        else:
                            nc.vector.tensor_copy(ot[:, dsl], po)
                    nc.sync.dma_start(out=out_d[i * P:(i + 1) * P, :], in_=ot)

            # --- pass 2: Q projections (chunk-major, hT now resident),
            # RoPE inline ---
            def emit_qchunk(t, c):
                csl = slice(c * CH, (c + 1) * CH)
                psq = psp.tile([P, CH], dt.float32, tag="sc", bufs=4,
                               name="ps_q")
                for k in range(NT):
                    nc.tensor.matmul(psq, wq[:, k, t * P:(t + 1) * P],
                                     hT[:, k, csl],
                                     start=(k == 0), stop=(k == NT - 1))
                pair_c = phbw.tile([P, CH], dt.bfloat16, tag="pair")
                nc.vector.tensor_copy(pair_c, psq)
                pmq = psp.tile([P, CH], dt.float32, tag="od", bufs=2)
                nc.tensor.matmul(pmq, pqm, pair_c, start=True, stop=True)
                t1 = phbw.tile([P, CH], dt.bfloat16, tag="t1")
                nc.vector.tensor_mul(t1, pair_c, cos2q[:, csl])
                t2 = phbw.tile([P, CH], dt.bfloat16, tag="t2")
                nc.vector.tensor_mul(t2, pmq, sin2q[:, csl])
                nc.vector.tensor_add(qt_pair[t][:, csl], t1, t2)

            # t=0 pass with inline score blocks: exp() starts as soon as
            # the first roped q-chunk exists, ~40us earlier than emitting
            # scores after the full pass
            emit_qchunk(0, 1)
            emit_scores(1, 0)
            emit_qchunk(0, 2)
            emit_scores(2, 0)
            emit_qchunk(0, 3)
            emit_qchunk(0, 0)
            # gate scaling + denominator column; emitted after the t=0
            # ropes so the wcol DRAM bounce never stalls them
            for st in range(NT):
                nc.vector.tensor_scalar(vaug[:, st, 0:HD], vraw[:, st, :],
                                        wcol[:, st:st + 1], None,
                                        mybir.AluOpType.mult)
                nc.vector.tensor_copy(vaug[:, st, HD:HD + 1],
                                      wcol[:, st:st + 1])

            # t=1 pass interleaved with the attention schedule.  AV units
            # consume score units in emission order (pt tags have 2 bufs),
            # each chunk's oproj trails its second AV pair by >=1 unit so
            # the DVE normalize chain never stalls the in-order PE, and
            # the cheapest chunk (c=0) lands last to shrink the tail.
            emit_qchunk(1, 1)
            emit_av(1, 0)
            emit_scores(1, 1)
            emit_qchunk(1, 2)
            emit_av(2, 0)
            emit_scores(3, 0)
            emit_qchunk(1, 3)
            emit_av(1, 1)
            emit_scores(2, 1)
            emit_qchunk(1, 0)
            emit_oproj(1, act_dc=0)
            emit_av(3, 0)
            emit_scores(3, 1)
            emit_av(2, 1)
            emit_scores(0, 0)
            emit_oproj(2, act_dc=1)
            emit_av(3, 1)
            emit_scores(0, 1)
            emit_oproj(3, act_dc=2)
            emit_av(0, 0)
            emit_av(0, 1)
            emit_oproj(0, act_dc=4)

    _split_sync_waits(nc)
    return nc


def kernel(**inputs):
    global LAST_RESULT
    inp = {k: np.asarray(v) for k, v in inputs.items()}
    h = inp["hidden_states"].astype(F32).reshape(S, D)
    mask = inp["attention_mask"].astype(F32).reshape(S, S)
    cos = inp["cos"].astype(F32)
    sin = inp["sin"].astype(F32)
    Wf = inp["Wf"].astype(F32)
    W1 = inp["W1"].astype(F32)
    b1 = inp["b1"].astype(F32)
    W2 = inp["W2"].astype(F32)
    b2 = float(inp["b2"].reshape(-1)[0])
    gate_scale = float(inp["gate_scale"])
    Wq = inp["Wq"].astype(F32)
    Wk = inp["Wk"].astype(F32)
    Wv = inp["Wv"].astype(F32)
    Wo = inp["Wo"].astype(F32)

    maskT = np.ascontiguousarray(mask.T)
    mb, patterns, av_incl = _analyze_mask(maskT)
    n_pat = len(patterns)
    assert n_pat <= 64, f"too many unique mask patterns ({n_pat})"

    field_scale = float(F32(1.0 - ALPHA))
    b2_scaled = float(F32(b2) * F32(field_scale))

    nc = _build_program(mb, n_pat, av_incl, field_scale, b2_scaled, gate_scale)

    # host-side shared tensors
    hT = np.ascontiguousarray(h.T).astype(BF16)
    cosT = np.ascontiguousarray(cos.T)                       # [64, S]
    sinT = np.ascontiguousarray(sin.T)
    sin_signed = sinT.copy()
    sin_signed[0:32] = -sin_signed[0:32]
    inv_sqrt_hd = 1.0 / math.sqrt(HD)
    cos2q = np.vstack([cosT, cosT]) * inv_sqrt_hd            # [128, S]
    sin2q = np.vstack([sin_signed, sin_signed]) * inv_sqrt_hd
    cosk = cosT.astype(BF16)
    sink = sin_signed.astype(BF16)
    # rotate-half permutation (swap 32-row halves within each 64-row head)
    pq = np.zeros((P, P), dtype=BF16)
    for m in range(P):
        base = (m // HD) * HD
        r = m - base
        src = base + (r + 32) % HD
        pq[src, m] = 1.0
    w1a = (W1[:D].astype(np.float64)
           + Wf.astype(np.float64) @ W1[D:].astype(np.float64)).astype(F32).astype(BF16)

    def kmajor(w):
        # [D, F] -> [128, NT*F]: partition-major so the SBUF load is contiguous
        f = w.shape[1]
        return np.ascontiguousarray(
            w.reshape(NT, P, f).transpose(1, 0, 2).reshape(P, NT * f))

    w1a = kmajor(w1a)
    sel2_host = np.zeros((HD + 1, P), dtype=BF16)
    sel2_host[0, 0:HD] = 1.0
    sel2_host[HD, HD:P] = 1.0
    w2 = W2.reshape(64, 1).astype(BF16)
    b1c = b1.reshape(64, 1).astype(F32)
    pm = np.stack(patterns) if n_pat else None

    in_maps = []
    for c in range(NCORES):
        m = {
            "hT": hT,
            "wq": kmajor(Wq[:, c * HLOC * HD:(c + 1) * HLOC * HD].astype(BF16)),
            "wkv": kmajor(np.concatenate(
                [Wk[:, c * HD:(c + 1) * HD], Wv[:, c * HD:(c + 1) * HD]],
                axis=1).astype(BF16)),
            "wo": np.ascontiguousarray(
                Wo[c * HLOC * HD:(c + 1) * HLOC * HD, :].astype(BF16)
                .reshape(2, P, D).transpose(1, 0, 2).reshape(P, 2 * D)),
            "w1a": w1a, "w2": w2, "b1": b1c,
            "cos2q": cos2q.astype(BF16), "sin2q": sin2q.astype(BF16),
            "cosk": cosk, "sink": sink, "pq": pq, "sel2": sel2_host,
        }
        if n_pat:
            m["pmask"] = pm
        in_maps.append(m)

    trace = False
    if os.environ.get("KERNEL_TRACE"):
        try:
            import antenv.axon_hooks  # noqa: F401  (profiling shim, dev only)
            trace = True
        except ImportError:
            pass

    res = run_bass_kernel_spmd(nc, in_maps, list(range(NCORES)), trace=trace)
    LAST_RESULT = res

    out = np.zeros((S, D), dtype=F32)
    for c in range(NCORES):
        out += res.results[c]["out"].astype(F32)
    return out.reshape(1, S, D)


# revision 53
# speedup vs baseline: 1.0094x; 1.0094x over previous
"""Trainium2 Bass kernel for nn_CFHoTWrapper (sparse attention with adapter gate).

Sharding: tensor-parallel over attention heads across 8 NeuronCores.
Each core computes 4 query heads + its 1 KV head end-to-end (QKV proj,
RoPE, scores, softmax, AV, partial O-projection); the tiny adapter gate
is replicated on every core. Per-core partial outputs (bf16) are summed
on the host.

Softmax is computed without max-subtraction (scores are O(5) for these
shapes so exp() is safe in fp32), and the per-key gate bias is folded in
multiplicatively: exp(s + m + g[k]) = exp(s) * exp(m) * w[k] with
w = exp(gate_scale * gate).  w scales the V rows, and an extra all-w
column appended to V yields the softmax denominator from the same
matmul that computes the numerator.

AV is computed TRANSPOSED: stationary = augmented V block [keys, 65],
moving = exp'd score tile [keys, q-chunk].  PSUM accumulates over key
blocks with partial-width matmuls (per-element has_written handles the
causal staircase).  This kills the per-(i,j) LDWEIGHTS reloads of the
score tiles (the old AV was weight-load-bound) AND produces attn^T
directly in the [head_dim, q] layout the O-projection needs as its
stationary operand - no PE transposes, no aT copies.  The softmax
denominator lands in psum row 64; its reciprocal row is broadcast to
64 partitions with a K=1 fp32r ones-matmul and folded into the
psum->SBUF cast.

RoPE's rotate-half is a fixed row permutation, done as a PE matmul with
a permutation matrix (sign folded into the sin table) instead of
SBUF->SBUF DMA.  The adapter projection is column-tiled: even k-slices
land in psum partitions 0-63 (tile_position (0,0)), odd slices in
64-127 ((0,64)), so pairs of slices stream concurrently through the PE
and the two half-sums are merged by one DVE scalar_tensor_tensor.

DMA order keeps both HWDGE queues streaming hT from instruction 0
(small early weights first, all later-phase weights behind the hT
stream) so the PE's k-outer passes never starve and TRN2's HAM clock
gate stays at full rate.
"""

import math
import os
from contextlib import ExitStack

import numpy as np
import ml_dtypes

import concourse.bass as bass
import concourse.tile as tile
from concourse import mybir
from concourse.masks import make_identity
from concourse.bass_utils import run_bass_kernel_spmd

BF16 = ml_dtypes.bfloat16
F32 = np.float32

S = 2048
D = 2048
HD = 64
NH = 32
NKV = 8
NCORES = 8
HLOC = NH // NCORES          # 4 query heads per core
P = 128
NT = S // P                  # 16 sequence tiles of 128
NCH = 4                      # 4 sequence chunks of 512
CH = 512
ALPHA = 0.995
MASK_NEG_THRESH = -80.0      # exp() underflows to 0 below this

LAST_RESULT = None           # BassKernelResults of the last run (for test.py)


def _analyze_mask(maskT):
    """Classify [keys=128 x q=128] blocks of maskT and dedup non-trivial
    multiplicative (exp) mask patterns. maskT is [S, S] (keys, q)."""
    mb = [[None] * NT for _ in range(NT)]
    patterns = []
    pat_index = {}
    for j in range(NT):
        for i in range(NT):
            blk = maskT[j * P:(j + 1) * P, i * P:(i + 1) * P]
            if (blk < MASK_NEG_THRESH).all():
                mb[j][i] = 'skip'
            elif (blk == 0.0).all():
                mb[j][i] = 'plain'
            else:
                pat = np.exp(np.minimum(blk, 80.0)).astype(BF16)
                key = pat.tobytes()
                if key not in pat_index:
                    pat_index[key] = len(patterns)
                    patterns.append(pat)
                mb[j][i] = pat_index[key]
    av_incl = [[j for j in range(NT) if mb[j][i] != 'skip'] for i in range(NT)]
    return mb, patterns, av_incl


def _split_sync_waits(nc):
    """This walrus build supports only ONE embedded sync wait per
    instruction; hoist extra waits onto preceding sequencer NoOps."""
    for f in nc.m.functions:
        for bb in f.blocks:
            insts = bb.instructions
            idx = 0
            while idx < len(insts):
                inst = insts[idx]
                si = inst.sync_info
                if si is not None and si.on_wait and len(si.on_wait) > 1:
                    waits = list(si.on_wait)
                    for w in waits[:-1]:
                        nop = mybir.InstNoOp(
                            name=nc.get_next_instruction_name(),
                            engine=inst.engine,
                            sync_info=mybir.SyncInfo(on_wait=[w], on_update=[]),
                            bass_nofuse=True,
                        )
                        nc.register_instruction(nop)
                        insts.insert(idx, nop)
                        idx += 1
                    inst.sync_info = mybir.SyncInfo(
                        on_wait=[waits[-1]], on_update=list(si.on_update))
                idx += 1


def _build_program(mb, n_pat, av_incl, field_scale, b2_scaled, gate_scale):
    nc = bass.Bass()
    dt = mybir.dt

    hT_d = nc.declare_dram_parameter("hT", [D, S], dt.bfloat16, isOutput=False)
    wq_d = nc.declare_dram_parameter("wq", [P, NT * HLOC * HD], dt.bfloat16, isOutput=False)
    wkv_d = nc.declare_dram_parameter("wkv", [P, NT * 2 * HD], dt.bfloat16, isOutput=False)
    wo_d = nc.declare_dram_parameter("wo", [P, 2 * D], dt.bfloat16, isOutput=False)
    w1a_d = nc.declare_dram_parameter("w1a", [P, NT * 64], dt.bfloat16, isOutput=False)
    w2_d = nc.declare_dram_parameter("w2", [64, 1], dt.bfloat16, isOutput=False)
    b1_d = nc.declare_dram_parameter("b1", [64, 1], dt.float32, isOutput=False)
    cos2q_d = nc.declare_dram_parameter("cos2q", [P, S], dt.bfloat16, isOutput=False)
    sin2q_d = nc.declare_dram_parameter("sin2q", [P, S], dt.bfloat16, isOutput=False)
    cosk_d = nc.declare_dram_parameter("cosk", [HD, S], dt.bfloat16, isOutput=False)
    sink_d = nc.declare_dram_parameter("sink", [HD, S], dt.bfloat16, isOutput=False)
    pq_d = nc.declare_dram_parameter("pq", [P, P], dt.bfloat16, isOutput=False)
    sel2_d = nc.declare_dram_parameter("sel2", [HD + 1, P], dt.bfloat16, isOutput=False)
    if n_pat:
        pm_d = nc.declare_dram_parameter("pmask", [n_pat, P, P], dt.bfloat16, isOutput=False)
    out_d = nc.declare_dram_parameter("out", [S, D], dt.bfloat16, isOutput=True)

    with tile.TileContext(nc) as tc, ExitStack() as ctx:
        pers = ctx.enter_context(tc.tile_pool(name="pers", bufs=1))
        psp = ctx.enter_context(tc.tile_pool(name="psum", bufs=4, space="PSUM"))

        # persistent (phase-C-lifetime) tiles; DMAs are emitted inside the
        # phb block below so small early-needed weights go first on the queue
        w2 = pers.tile([64, 1], dt.bfloat16)
        b1 = pers.tile([64, 1], dt.float32)
        pqm = pers.tile([P, P], dt.bfloat16)
        # hT parity-major: [:, k%2, k//2, :] so one rearranged DMA per
        # (chunk, parity) delivers a whole column chunk of 8 k-slices
        hT2 = pers.tile([P, 2, NT // 2, S], dt.bfloat16)

        def ht(k):
            return hT2[:, k % 2, k // 2, :]
        wq = pers.tile([P, NT, HLOC * HD], dt.bfloat16)
        if n_pat:
            pmask = pers.tile([P, n_pat, P], dt.bfloat16)
        wo = pers.tile([P, 2, D], dt.bfloat16)
        ident = pers.tile([P, P], dt.bfloat16)
        make_identity(nc, ident)
        # selector for the denominator broadcast: contraction row 0 -> ones
        # on out rows 0-63 (head A), row 64 -> ones on 64-127 (head B); all
        # other rows are zero, so one K=65 matmul fans both rows out
        sel2 = pers.tile([HD + 1, P], dt.bfloat16)


        qt_pair = [pers.tile([P, S], dt.bfloat16, tag=f"qp{t}", name=f"qp{t}")
                   for t in range(HLOC // 2)]
        kt = pers.tile([HD, S], dt.bfloat16)
        ktp = pers.tile([P, S], dt.bfloat16)      # kt duplicated at base 64
        vaug = pers.tile([P, NT, HD + 1], dt.bfloat16)
        wcol = pers.tile([P, NT], dt.float32)
        cos2q = pers.tile([P, S], dt.bfloat16)
        sin2q = pers.tile([P, S], dt.bfloat16)

        phbw = ctx.enter_context(tc.tile_pool(name="phbw", bufs=2))
        with tc.tile_pool(name="phb", bufs=1) as phb:
            # ---- load order: tiny early weights lead, then hT arrives
            # CHUNK-major (one rearranged DMA per chunk x parity, evens on
            # SP / odds on ACT) so each 512-column chunk completes early
            # and the whole per-chunk pipeline (KV+A proj, gelu, K-RoPE,
            # V transpose) streams behind it; later-phase weights are
            # slotted by their first-use time ----
            w1a = phb.tile([P, NT, 64], dt.bfloat16)
            nc.scalar.dma_start(out=w1a, in_=w1a_d[:, :])
            wkv = phb.tile([P, NT, 2 * HD], dt.bfloat16)
            nc.scalar.dma_start(out=wkv, in_=wkv_d[:, :])
            nc.sync.dma_start(out=w2, in_=w2_d[:, :])
            nc.sync.dma_start(out=b1, in_=b1_d[:, :])
            nc.sync.dma_start(out=pqm, in_=pq_d[:, :])
            nc.sync.dma_start(out=sel2, in_=sel2_d[:, :])
            cosk = phb.tile([HD, S], dt.bfloat16)
            nc.sync.dma_start(out=cosk, in_=cosk_d[:, :])
            sink = phb.tile([HD, S], dt.bfloat16)
            nc.sync.dma_start(out=sink, in_=sink_d[:, :])

            def ht_chunk_dma(c):
                csl = slice(c * CH, (c + 1) * CH)
                r = hT_d[:, csl].rearrange("(k two p) c -> two p k c",
                                           two=2, p=P)
                nc.sync.dma_start(out=hT2[:, 0, :, csl], in_=r[0])
                nc.scalar.dma_start(out=hT2[:, 1, :, csl], in_=r[1])

            ht_chunk_dma(0)
            ht_chunk_dma(1)
            nc.scalar.dma_start(out=wq, in_=wq_d[:, :])
            nc.sync.dma_start(out=cos2q, in_=cos2q_d[:, :])
            nc.scalar.dma_start(out=sin2q, in_=sin2q_d[:, :])
            ht_chunk_dma(2)
            if n_pat:
                for m in range(n_pat):
                    nc.sync.dma_start(out=pmask[:, m, :], in_=pm_d[m, :, :])
            ht_chunk_dma(3)
            nc.sync.dma_start(out=wo, in_=wo_d[:, :])

            # --- per-chunk pipeline trailing the chunk-major hT stream:
            # KV first (scores critical path), adapter, gelu, K-RoPE,
            # V transposes ---
            hmT = phb.tile([64, S], dt.bfloat16)
            kraw = phb.tile([HD, S], dt.bfloat16)
            vt = phb.tile([HD, S], dt.bfloat16)
            vraw = phbw.tile([P, NT, HD], dt.bfloat16, tag="vraw", bufs=1)
            for c in range(NCH):
                csl = slice(c * CH, (c + 1) * CH)
                accKV = psp.tile([P, CH], dt.float32, tag="sc", bufs=4,
                                 name=f"accKV{c}")
                for k in range(NT):
                    nc.tensor.matmul(accKV, wkv[:, k, :], ht(k)[:, csl],
                                     start=(k == 0), stop=(k == NT - 1))
                accA = psp.tile([64, CH], dt.float32, tag="sc", bufs=4,
                                name=f"accA{c}")
                for k in range(NT):
                    nc.tensor.matmul(accA, w1a[:, k, :], ht(k)[:, csl],
                                     start=(k == 0), stop=(k == NT - 1))
                nc.vector.tensor_copy(kraw[:, csl], accKV[0:HD, :])
                nc.vector.tensor_copy(vt[:, csl], accKV[HD:P, :])
                # K RoPE: rotate-half via PE permutation matmul
                pmk = psp.tile([HD, CH], dt.float32, tag="od", bufs=2)
                nc.tensor.matmul(pmk, pqm[0:HD, 0:HD], kraw[:, csl],
                                 start=True, stop=True)
                t1k = phbw.tile([HD, CH], dt.bfloat16, tag="t1")
                nc.vector.tensor_mul(t1k, kraw[:, csl], cosk[:, csl])
                t2k = phbw.tile([HD, CH], dt.bfloat16, tag="t2")
                nc.vector.tensor_mul(t2k, pmk, sink[:, csl])
                nc.vector.tensor_add(kt[:, csl], t1k, t2k)
                nc.sync.dma_start(out=ktp[HD:P, csl], in_=kt[:, csl])
                # exact gelu(x) = 0.5 * x * (1 + erf(x / sqrt(2))), x = ps + b1
                pre = phbw.tile([64, CH], dt.float32, tag="pre")
                nc.vector.tensor_scalar(pre, accA, b1, None, mybir.AluOpType.add)
                er = phbw.tile([64, CH], dt.float32, tag="er")
                nc.scalar.activation(er, pre, mybir.ActivationFunctionType.Erf,
                                     bias=0.0, scale=1.0 / math.sqrt(2.0))
                nc.vector.tensor_scalar(er, er, 0.5, 0.5,
                                        mybir.AluOpType.mult, mybir.AluOpType.add)
                nc.vector.tensor_mul(hmT[:, csl], pre, er)
                # V tiles: PE transpose into unscaled vraw (the wcol gate
                # scaling runs later so the slow field chain never stalls
                # the in-order PE queue)
                for st in range(4 * c, 4 * c + 4):
                    pv = psp.tile([P, HD], dt.bfloat16, tag="od", bufs=2)
                    nc.tensor.transpose(pv, vt[:, st * P:(st + 1) * P],
                                        ident[0:HD, 0:HD])
                    nc.vector.tensor_copy(vraw[:, st, :], pv)

            # --- field row = field_scale * (hmidT^T @ W2 + b2); gate ---
            field = phb.tile([1, S], dt.float32)
            scratch = phb.tile([1, S], dt.float32)
            for c in range(NCH):
                ps = psp.tile([1, CH], dt.float32, tag="sc", bufs=4)
                nc.tensor.matmul(ps, w2, hmT[:, c * CH:(c + 1) * CH],
                                 start=True, stop=True)
                nc.vector.tensor_scalar(field[:, c * CH:(c + 1) * CH], ps,
                                        field_scale, b2_scaled,
                                        mybir.AluOpType.mult, mybir.AluOpType.add)
            ssum = phb.tile([1, 1], dt.float32)
            nc.vector.reduce_sum(ssum, field, axis=mybir.AxisListType.X)
            mean = phb.tile([1, 1], dt.float32)
            nc.vector.tensor_scalar_mul(mean, ssum, 1.0 / S)
            nc.vector.tensor_scalar(field, field, mean, None, mybir.AluOpType.subtract)
            nc.scalar.square(scratch, field)
            ss2 = phb.tile([1, 1], dt.float32)
            nc.vector.reduce_sum(ss2, scratch, axis=mybir.AxisListType.X)
            std = phb.tile([1, 1], dt.float32)
            nc.scalar.activation(std, ss2, mybir.ActivationFunctionType.Sqrt,
                                 bias=0.0, scale=1.0 / (S - 1))
            nc.vector.tensor_scalar_add(std, std, 1e-6)
            rstd = phb.tile([1, 1], dt.float32)
            nc.vector.reciprocal(rstd, std)
            gsr = phb.tile([1, 1], dt.float32)
            nc.vector.tensor_scalar_mul(gsr, rstd, gate_scale)
            # w row = exp(gate_scale * gate), into scratch
            nc.scalar.activation(scratch, field, mybir.ActivationFunctionType.Exp,
                                 bias=0.0, scale=gsr)
            # transpose the w row into per-partition columns [128, 16] via a
            # DRAM bounce (SBUF partitions are not element-addressable across
            # the partition stride, so an in-SBUF gather is illegal on HW).
            # Use the gpsimd SWDGE queue: independent of the two HWDGE rings
            # so the bounce never queues behind bulk weight traffic.
            wrow_dram = nc.dram_tensor("wrow_dram", [1, S], dt.float32)
            nc.gpsimd.dma_start(out=wrow_dram[:, :], in_=scratch)
            nc.gpsimd.dma_start(out=wcol,
                                in_=wrow_dram[0, :].rearrange("(j p) -> p j", p=P))


        # ------------- phase C setup: attention emit helpers -------------
        with tc.tile_pool(name="phc", bufs=2) as phc, \
             tc.tile_pool(name="phcs", bufs=4) as phcs, \
             tc.tile_pool(name="phd", bufs=2) as phd:

            def chunk_start(j, c):
                for ii in range(4 * c, 4 * c + 4):
                    if mb[j][ii] != 'skip':
                        return (ii % 4) * P
                return None

            pts_store = {}
            aT_store = {}

            def emit_scores(c, t):
                """Scores for head pair (2t, 2t+1): K=64 matmuls in PE
                row-groups 0 and 64 (A at base 0, B at base 64)."""
                ptsA, ptsB = {}, {}
                for j in range(NT):
                    s0 = chunk_start(j, c)
                    if s0 is None:
                        continue
                    jsl = slice(j * P, (j + 1) * P)
                    csl = slice(c * CH + s0, (c + 1) * CH)
                    psA = psp.tile([P, CH], dt.float32, tag="sc", bufs=4,
                                   name="ps_scA")
                    nc.tensor.matmul(psA[:, s0:CH], kt[:, jsl],
                                     qt_pair[t][0:HD, csl],
                                     start=True, stop=True, tile_position=(0, 0))
                    psB = psp.tile([P, CH], dt.float32, tag="sc", bufs=4,
                                   name="ps_scB")
                    nc.tensor.matmul(psB[:, s0:CH], ktp[HD:P, jsl],
                                     qt_pair[t][HD:P, csl],
                                     start=True, stop=True, tile_position=(64, 0))
                    for pts, ps, tagc in ((ptsA, psA, "pt"), (ptsB, psB, "pu")):
                        pt = phc.tile([P, CH], dt.bfloat16, tag=f"{tagc}{j}",
                                      name=f"{tagc}{j}")
                        nc.scalar.activation(pt[:, s0:CH], ps[:, s0:CH],
                                             mybir.ActivationFunctionType.Exp)
                        for ii in range(4 * c, 4 * c + 4):
                            kind = mb[j][ii]
                            if kind in ('skip', 'plain'):
                                continue
                            qq = slice((ii % 4) * P, (ii % 4 + 1) * P)
                            nc.vector.tensor_mul(pt[:, qq], pt[:, qq],
                                                 pmask[:, kind, :])
                        pts[j] = (pt, s0)
                pts_store[(c, 2 * t)] = ptsA
                pts_store[(c, 2 * t + 1)] = ptsB

            def emit_av(c, t):
                """Transposed AV for head pair t of chunk c: for each head,
                one psum [65, CH] accumulates stationary-V matmuls over key
                blocks (partial widths ride the per-element has_written
                bits).  Row 64 is the softmax denominator; its reciprocal
                row is PE-broadcast to 64 partitions and folded into the
                psum->SBUF cast, which writes attn^T for the head pair
                stacked [128, CH] - the O-projection stationary."""
                aTu = phcs.tile([P, CH], dt.bfloat16, tag="aTu", bufs=3,
                                name="aTu")
                den65 = phcs.tile([HD + 1, CH], dt.bfloat16, tag="den65",
                                  bufs=1, name="den65")
                # rows 1-63 feed the selector matmul with zero weights; they
                # must hold finite values (0 x NaN = NaN), so fill with 1.0
                nc.vector.memset(den65, 1.0)
                for hh in range(2):
                    h = 2 * t + hh
                    pts = pts_store.pop((c, h))
                    js = sorted(pts.keys())
                    pavT = psp.tile([HD + 1, CH], dt.float32, tag="av", bufs=2,
                                    name="ps_avT")
                    assert js and pts[js[0]][1] == 0, "first key block must span chunk"
                    for idx, j in enumerate(js):
                        pt, s0 = pts[j]
                        nc.tensor.matmul(pavT[:, s0:CH], vaug[:, j, :],
                                         pt[:, s0:CH],
                                         start=(idx == 0),
                                         stop=(idx == len(js) - 1))
                    # free the psum bank right away: numerators to SBUF
                    # (heads stacked), denominator row to partition hh*64
                    nc.vector.tensor_copy(aTu[hh * HD:(hh + 1) * HD, :],
                                          pavT[0:HD, :])
                    nc.vector.tensor_copy(den65[hh * HD:hh * HD + 1, :],
                                          pavT[HD:HD + 1, :])
                # one batched reciprocal covers both heads' denominator rows;
                # the broadcast + normalize run later (in this chunk's oproj
                # unit) so this slow DVE op is never on the PE critical path
                rcr65 = phcs.tile([HD + 1, CH], dt.bfloat16, tag="rcr65",
                                  bufs=3, name="rcr65")
                with nc.allow_low_precision(
                        reason="bf16 softmax-denominator reciprocal: 0.4% "
                               "per-query scale, inside the error budget"):
                    nc.vector.reciprocal(rcr65, den65)
                aT_store[(c, t)] = (aTu, rcr65)

            def emit_oproj(c, act_dc=0):
                # deferred normalize: the reciprocals were computed an entire
                # schedule unit ago, so the selector matmuls fire without
                # stalling the PE; then attn^T = unnormalized x broadcast rcp
                aTs = []
                parts = [aT_store.pop((c, t)) for t in range(HLOC // 2)]
                rcbs_t = []
                for t in range(HLOC // 2):
                    rcb = psp.tile([P, CH], dt.float32, tag="od", bufs=2,
                                   name="ps_rcb")
                    nc.tensor.matmul(rcb, sel2, parts[t][1],
                                     start=True, stop=True)
                    rcbs_t.append(rcb)
                for t in range(HLOC // 2):
                    rcbs = phcs.tile([P, CH], dt.bfloat16, tag="rcbs", bufs=2,
                                     name="rcbs")
                    nc.vector.tensor_copy(rcbs, rcbs_t[t])
                    aTt = phcs.tile([P, CH], dt.bfloat16, tag=f"aT{t}",
                                    bufs=1, name=f"aT{t}")
                    nc.vector.tensor_mul(aTt, parts[t][0], rcbs)
                    aTs.append(aTt)
                for i in range(4 * c, 4 * c + 4):
                    qsl = slice((i % 4) * P, (i % 4 + 1) * P)
                    ot = phd.tile([P, D], dt.bfloat16, tag="outsb", name="ot")
                    for dc in range(NCH):
                        po = psp.tile([P, CH], dt.float32, tag="od", bufs=2,
                                      name="ps_o")
                        dsl = slice(dc * CH, (dc + 1) * CH)
                        nc.tensor.matmul(po, aTs[0][:, qsl], wo[:, 0, dsl],
                                         start=True, stop=False)
                        nc.tensor.matmul(po, aTs[1][:, qsl], wo[:, 1, dsl],
                                         start=False, stop=True)
                        if dc < act_dc:
                            nc.scalar.copy(ot[:, dsl], po)
                        else:
                            nc.vector.tensor_copy(ot[:, dsl], po)
                    nc.sync.dma_start(out=out_d[i * P:(i + 1) * P, :], in_=ot)

            # --- pass 2: Q projections (chunk-major, hT now resident),
            # RoPE inline ---
            def emit_qchunk(t, c):
                csl = slice(c * CH, (c + 1) * CH)
                psq = psp.tile([P, CH], dt.float32, tag="sc", bufs=4,
                               name="ps_q")
                for k in range(NT):
                    nc.tensor.matmul(psq, wq[:, k, t * P:(t + 1) * P],
                                     ht(k)[:, csl],
                                     start=(k == 0), stop=(k == NT - 1))
                pair_c = phbw.tile([P, CH], dt.bfloat16, tag="pair")
                nc.vector.tensor_copy(pair_c, psq)
                pmq = psp.tile([P, CH], dt.float32, tag="od", bufs=2)
                nc.tensor.matmul(pmq, pqm, pair_c, start=True, stop=True)
                t1 = phbw.tile([P, CH], dt.bfloat16, tag="t1")
                nc.vector.tensor_mul(t1, pair_c, cos2q[:, csl])
                t2 = phbw.tile([P, CH], dt.bfloat16, tag="t2")
                nc.vector.tensor_mul(t2, pmq, sin2q[:, csl])
                nc.vector.tensor_add(qt_pair[t][:, csl], t1, t2)

            # t=0 pass with inline score blocks: exp() starts as soon as
            # the first roped q-chunk exists, ~40us earlier than emitting
            # scores after the full pass
            emit_qchunk(0, 1)
            emit_scores(1, 0)
            emit_qchunk(0, 2)
            emit_scores(2, 0)
            emit_qchunk(0, 3)
            emit_qchunk(0, 0)
            # gate scaling + denominator column; emitted after the t=0
            # ropes so the wcol DRAM bounce never stalls them
            for st in range(NT):
                nc.vector.tensor_scalar(vaug[:, st, 0:HD], vraw[:, st, :],
                                        wcol[:, st:st + 1], None,
                                        mybir.AluOpType.mult)
                nc.vector.tensor_copy(vaug[:, st, HD:HD + 1],
                                      wcol[:, st:st + 1])

            # t=1 pass interleaved with the attention schedule.  AV units
            # consume score units in emission order (pt tags have 2 bufs),
            # each chunk's oproj trails its second AV pair by >=1 unit so
            # the DVE normalize chain never stalls the in-order PE, and
            # the cheapest chunk (c=0) lands last to shrink the tail.
            emit_qchunk(1, 1)
            emit_av(1, 0)
            emit_scores(1, 1)
            emit_qchunk(1, 2)
            emit_av(2, 0)
            emit_scores(3, 0)
            emit_qchunk(1, 3)
            emit_av(1, 1)
            emit_scores(2, 1)
            emit_qchunk(1, 0)
            emit_oproj(1, act_dc=0)
            emit_av(3, 0)
            emit_scores(3, 1)
            emit_av(2, 1)
            emit_scores(0, 0)
            emit_oproj(2, act_dc=1)
            emit_av(3, 1)
            emit_scores(0, 1)
            emit_oproj(3, act_dc=2)
            emit_av(0, 0)
            emit_av(0, 1)
            emit_oproj(0, act_dc=4)

    _split_sync_waits(nc)
    return nc


def kernel(**inputs):
    global LAST_RESULT
    inp = {k: np.asarray(v) for k, v in inputs.items()}
    h = inp["hidden_states"].astype(F32).reshape(S, D)
    mask = inp["attention_mask"].astype(F32).reshape(S, S)
    cos = inp["cos"].astype(F32)
    sin = inp["sin"].astype(F32)
    Wf = inp["Wf"].astype(F32)
    W1 = inp["W1"].astype(F32)
    b1 = inp["b1"].astype(F32)
    W2 = inp["W2"].astype(F32)
    b2 = float(inp["b2"].reshape(-1)[0])
    gate_scale = float(inp["gate_scale"])
    Wq = inp["Wq"].astype(F32)
    Wk = inp["Wk"].astype(F32)
    Wv = inp["Wv"].astype(F32)
    Wo = inp["Wo"].astype(F32)

    maskT = np.ascontiguousarray(mask.T)
    mb, patterns, av_incl = _analyze_mask(maskT)
    n_pat = len(patterns)
    assert n_pat <= 64, f"too many unique mask patterns ({n_pat})"

    field_scale = float(F32(1.0 - ALPHA))
    b2_scaled = float(F32(b2) * F32(field_scale))

    nc = _build_program(mb, n_pat, av_incl, field_scale, b2_scaled, gate_scale)

    # host-side shared tensors
    hT = np.ascontiguousarray(h.T).astype(BF16)
    cosT = np.ascontiguousarray(cos.T)                       # [64, S]
    sinT = np.ascontiguousarray(sin.T)
    sin_signed = sinT.copy()
    sin_signed[0:32] = -sin_signed[0:32]
    inv_sqrt_hd = 1.0 / math.sqrt(HD)
    cos2q = np.vstack([cosT, cosT]) * inv_sqrt_hd            # [128, S]
    sin2q = np.vstack([sin_signed, sin_signed]) * inv_sqrt_hd
    cosk = cosT.astype(BF16)
    sink = sin_signed.astype(BF16)
    # rotate-half permutation (swap 32-row halves within each 64-row head)
    pq = np.zeros((P, P), dtype=BF16)
    for m in range(P):
        base = (m // HD) * HD
        r = m - base
        src = base + (r + 32) % HD
        pq[src, m] = 1.0
    w1a = (W1[:D].astype(np.float64)
           + Wf.astype(np.float64) @ W1[D:].astype(np.float64)).astype(F32).astype(BF16)

    def kmajor(w):
        # [D, F] -> [128, NT*F]: partition-major so the SBUF load is contiguous
        f = w.shape[1]
        return np.ascontiguousarray(
            w.reshape(NT, P, f).transpose(1, 0, 2).reshape(P, NT * f))

    w1a = kmajor(w1a)
    sel2_host = np.zeros((HD + 1, P), dtype=BF16)
    sel2_host[0, 0:HD] = 1.0
    sel2_host[HD, HD:P] = 1.0
    w2 = W2.reshape(64, 1).astype(BF16)
    b1c = b1.reshape(64, 1).astype(F32)
    pm = np.stack(patterns) if n_pat else None

    in_maps = []
    for c in range(NCORES):
        m = {
            "hT": hT,
            "wq": kmajor(Wq[:, c * HLOC * HD:(c + 1) * HLOC * HD].astype(BF16)),
            "wkv": kmajor(np.concatenate(
                [Wk[:, c * HD:(c + 1) * HD], Wv[:, c * HD:(c + 1) * HD]],
                axis=1).astype(BF16)),
            "wo": np.ascontiguousarray(
                Wo[c * HLOC * HD:(c + 1) * HLOC * HD, :].astype(BF16)
                .reshape(2, P, D).transpose(1, 0, 2).reshape(P, 2 * D)),
            "w1a": w1a, "w2": w2, "b1": b1c,
            "cos2q": cos2q.astype(BF16), "sin2q": sin2q.astype(BF16),
            "cosk": cosk, "sink": sink, "pq": pq, "sel2": sel2_host,
        }
        if n_pat:
            m["pmask"] = pm
        in_maps.append(m)

    trace = False
    if os.environ.get("KERNEL_TRACE"):
        try:
            import antenv.axon_hooks  # noqa: F401  (profiling shim, dev only)
            trace = True
        except ImportError:
            pass

    res = run_bass_kernel_spmd(nc, in_maps, list(range(NCORES)), trace=trace)
    LAST_RESULT = res

    out = np.zeros((S, D), dtype=F32)
    for c in range(NCORES):
        out += res.results[c]["out"].astype(F32)
    return out.reshape(1, S, D)


# revision 59
# speedup vs baseline: 1.0223x; 1.0128x over previous
"""Trainium2 Bass kernel for nn_CFHoTWrapper (sparse attention with adapter gate).

Sharding: tensor-parallel over attention heads across 8 NeuronCores.
Each core computes 4 query heads + its 1 KV head end-to-end (QKV proj,
RoPE, scores, softmax, AV, partial O-projection); the tiny adapter gate
is replicated on every core. Per-core partial outputs (bf16) are summed
on the host.

Softmax is computed without max-subtraction (scores are O(5) for these
shapes so exp() is safe in fp32), and the per-key gate bias is folded in
multiplicatively: exp(s + m + g[k]) = exp(s) * exp(m) * w[k] with
w = exp(gate_scale * gate).  w scales the V rows, and an extra all-w
column appended to V yields the softmax denominator from the same
matmul that computes the numerator.

AV is computed TRANSPOSED: stationary = augmented V block [keys, 65],
moving = exp'd score tile [keys, q-chunk].  PSUM accumulates over key
blocks with partial-width matmuls (per-element has_written handles the
causal staircase).  This kills the per-(i,j) LDWEIGHTS reloads of the
score tiles (the old AV was weight-load-bound) AND produces attn^T
directly in the [head_dim, q] layout the O-projection needs as its
stationary operand - no PE transposes, no aT copies.  The softmax
denominator lands in psum row 64; its reciprocal row is broadcast to
64 partitions with a K=1 fp32r ones-matmul and folded into the
psum->SBUF cast.

RoPE's rotate-half is a fixed row permutation, done as a PE matmul with
a permutation matrix (sign folded into the sin table) instead of
SBUF->SBUF DMA.  The adapter projection is column-tiled: even k-slices
land in psum partitions 0-63 (tile_position (0,0)), odd slices in
64-127 ((0,64)), so pairs of slices stream concurrently through the PE
and the two half-sums are merged by one DVE scalar_tensor_tensor.

DMA order keeps both HWDGE queues streaming hT from instruction 0
(small early weights first, all later-phase weights behind the hT
stream) so the PE's k-outer passes never starve and TRN2's HAM clock
gate stays at full rate.
"""

import math
import os
from contextlib import ExitStack

import numpy as np
import ml_dtypes

import concourse.bass as bass
import concourse.tile as tile
from concourse import mybir
from concourse.masks import make_identity
from concourse.bass_utils import run_bass_kernel_spmd

BF16 = ml_dtypes.bfloat16
F32 = np.float32

S = 2048
D = 2048
HD = 64
NH = 32
NKV = 8
NCORES = 8
HLOC = NH // NCORES          # 4 query heads per core
P = 128
NT = S // P                  # 16 sequence tiles of 128
NCH = 4                      # 4 sequence chunks of 512
CH = 512
ALPHA = 0.995
MASK_NEG_THRESH = -80.0      # exp() underflows to 0 below this

LAST_RESULT = None           # BassKernelResults of the last run (for test.py)


def _analyze_mask(maskT):
    """Classify [keys=128 x q=128] blocks of maskT and dedup non-trivial
    multiplicative (exp) mask patterns. maskT is [S, S] (keys, q)."""
    mb = [[None] * NT for _ in range(NT)]
    patterns = []
    pat_index = {}
    for j in range(NT):
        for i in range(NT):
            blk = maskT[j * P:(j + 1) * P, i * P:(i + 1) * P]
            if (blk < MASK_NEG_THRESH).all():
                mb[j][i] = 'skip'
            elif (blk == 0.0).all():
                mb[j][i] = 'plain'
            else:
                pat = np.exp(np.minimum(blk, 80.0)).astype(BF16)
                key = pat.tobytes()
                if key not in pat_index:
                    pat_index[key] = len(patterns)
                    patterns.append(pat)
                mb[j][i] = pat_index[key]
    av_incl = [[j for j in range(NT) if mb[j][i] != 'skip'] for i in range(NT)]
    return mb, patterns, av_incl


def _split_sync_waits(nc):
    """This walrus build supports only ONE embedded sync wait per
    instruction; hoist extra waits onto preceding sequencer NoOps."""
    for f in nc.m.functions:
        for bb in f.blocks:
            insts = bb.instructions
            idx = 0
            while idx < len(insts):
                inst = insts[idx]
                si = inst.sync_info
                if si is not None and si.on_wait and len(si.on_wait) > 1:
                    waits = list(si.on_wait)
                    for w in waits[:-1]:
                        nop = mybir.InstNoOp(
                            name=nc.get_next_instruction_name(),
                            engine=inst.engine,
                            sync_info=mybir.SyncInfo(on_wait=[w], on_update=[]),
                            bass_nofuse=True,
                        )
                        nc.register_instruction(nop)
                        insts.insert(idx, nop)
                        idx += 1
                    inst.sync_info = mybir.SyncInfo(
                        on_wait=[waits[-1]], on_update=list(si.on_update))
                idx += 1


def _build_program(mb, n_pat, av_incl, field_scale, b2_scaled, gate_scale):
    nc = bass.Bass()
    dt = mybir.dt

    # hT pre-shuffled on the host to (chunk, parity, partition, kk, cols) so
    # each (chunk, parity) load is one plain 2D DMA with 8KB-contiguous rows
    hT_d = nc.declare_dram_parameter("hT", [NCH * 2 * P, (NT // 2) * CH],
                                     dt.bfloat16, isOutput=False)
    wq_d = nc.declare_dram_parameter("wq", [P, NT * HLOC * HD], dt.bfloat16, isOutput=False)
    wkv_d = nc.declare_dram_parameter("wkv", [P, NT * 2 * HD], dt.bfloat16, isOutput=False)
    wo_d = nc.declare_dram_parameter("wo", [P, 2 * D], dt.bfloat16, isOutput=False)
    w1a_d = nc.declare_dram_parameter("w1a", [P, NT * 64], dt.bfloat16, isOutput=False)
    w2_d = nc.declare_dram_parameter("w2", [64, 1], dt.bfloat16, isOutput=False)
    b1_d = nc.declare_dram_parameter("b1", [64, 1], dt.float32, isOutput=False)
    cos2q_d = nc.declare_dram_parameter("cos2q", [P, S], dt.bfloat16, isOutput=False)
    sin2q_d = nc.declare_dram_parameter("sin2q", [P, S], dt.bfloat16, isOutput=False)
    cosk_d = nc.declare_dram_parameter("cosk", [HD, S], dt.bfloat16, isOutput=False)
    sink_d = nc.declare_dram_parameter("sink", [HD, S], dt.bfloat16, isOutput=False)
    pq_d = nc.declare_dram_parameter("pq", [P, P], dt.bfloat16, isOutput=False)
    sel2_d = nc.declare_dram_parameter("sel2", [HD + 1, P], dt.bfloat16, isOutput=False)
    if n_pat:
        pm_d = nc.declare_dram_parameter("pmask", [n_pat, P, P], dt.bfloat16, isOutput=False)
    out_d = nc.declare_dram_parameter("out", [S, D], dt.bfloat16, isOutput=True)

    with tile.TileContext(nc) as tc, ExitStack() as ctx:
        pers = ctx.enter_context(tc.tile_pool(name="pers", bufs=1))
        psp = ctx.enter_context(tc.tile_pool(name="psum", bufs=4, space="PSUM"))

        # persistent (phase-C-lifetime) tiles; DMAs are emitted inside the
        # phb block below so small early-needed weights go first on the queue
        w2 = pers.tile([64, 1], dt.bfloat16)
        b1 = pers.tile([64, 1], dt.float32)
        pqm = pers.tile([P, P], dt.bfloat16)
        # hT chunk-major: [:, c, k%2, k//2, :] with the chunk dim outermost
        # so a (chunk, parity) load lands as one contiguous 8KB run per
        # partition (128 fat descriptors per DMA, ~0.6us issue cost)
        hT3 = pers.tile([P, NCH, 2, NT // 2, CH], dt.bfloat16)

        def ht(k, c):
            return hT3[:, c, k % 2, k // 2, :]
        wq = pers.tile([P, NT, HLOC * HD], dt.bfloat16)
        if n_pat:
            pmask = pers.tile([P, n_pat, P], dt.bfloat16)
        wo = pers.tile([P, 2, D], dt.bfloat16)
        ident = pers.tile([P, P], dt.bfloat16)
        make_identity(nc, ident)
        # selector for the denominator broadcast: contraction row 0 -> ones
        # on out rows 0-63 (head A), row 64 -> ones on 64-127 (head B); all
        # other rows are zero, so one K=65 matmul fans both rows out
        sel2 = pers.tile([HD + 1, P], dt.bfloat16)


        qt_pair = [pers.tile([P, S], dt.bfloat16, tag=f"qp{t}", name=f"qp{t}")
                   for t in range(HLOC // 2)]
        kt = pers.tile([HD, S], dt.bfloat16)
        ktp = pers.tile([P, S], dt.bfloat16)      # kt duplicated at base 64
        vaug = pers.tile([P, NT, HD + 1], dt.bfloat16)
        wcol = pers.tile([P, NT], dt.float32)
        cos2q = pers.tile([P, S], dt.bfloat16)
        sin2q = pers.tile([P, S], dt.bfloat16)

        phbw = ctx.enter_context(tc.tile_pool(name="phbw", bufs=2))
        with tc.tile_pool(name="phb", bufs=1) as phb:
            # ---- load order: tiny early weights lead, then hT arrives
            # CHUNK-major (one rearranged DMA per chunk x parity, evens on
            # SP / odds on ACT) so each 512-column chunk completes early
            # and the whole per-chunk pipeline (KV+A proj, gelu, K-RoPE,
            # V transpose) streams behind it; later-phase weights are
            # slotted by their first-use time ----
            w1a = phb.tile([P, NT, 64], dt.bfloat16)
            nc.scalar.dma_start(out=w1a, in_=w1a_d[:, :])
            wkv = phb.tile([P, NT, 2 * HD], dt.bfloat16)
            nc.scalar.dma_start(out=wkv, in_=wkv_d[:, :])
            nc.sync.dma_start(out=w2, in_=w2_d[:, :])
            nc.sync.dma_start(out=b1, in_=b1_d[:, :])
            nc.sync.dma_start(out=pqm, in_=pq_d[:, :])
            nc.sync.dma_start(out=sel2, in_=sel2_d[:, :])
            cosk = phb.tile([HD, S], dt.bfloat16)
            nc.sync.dma_start(out=cosk, in_=cosk_d[:, :])
            sink = phb.tile([HD, S], dt.bfloat16)
            nc.sync.dma_start(out=sink, in_=sink_d[:, :])

            def ht_chunk_dma(c):
                nc.sync.dma_start(out=hT3[:, c, 0, :, :],
                                  in_=hT_d[(2 * c) * P:(2 * c + 1) * P, :])
                nc.scalar.dma_start(out=hT3[:, c, 1, :, :],
                                    in_=hT_d[(2 * c + 1) * P:(2 * c + 2) * P, :])

            ht_chunk_dma(0)
            ht_chunk_dma(1)
            nc.scalar.dma_start(out=wq, in_=wq_d[:, :])
            nc.sync.dma_start(out=cos2q, in_=cos2q_d[:, :])
            nc.scalar.dma_start(out=sin2q, in_=sin2q_d[:, :])
            ht_chunk_dma(2)
            if n_pat:
                for m in range(n_pat):
                    nc.sync.dma_start(out=pmask[:, m, :], in_=pm_d[m, :, :])
            ht_chunk_dma(3)
            nc.sync.dma_start(out=wo, in_=wo_d[:, :])

            # --- per-chunk pipeline trailing the chunk-major hT stream:
            # KV first (scores critical path), adapter, gelu, K-RoPE,
            # V transposes ---
            hmT = phb.tile([64, S], dt.bfloat16)
            kraw = phb.tile([HD, S], dt.bfloat16)
            vt = phb.tile([HD, S], dt.bfloat16)
            vraw = phbw.tile([P, NT, HD], dt.bfloat16, tag="vraw", bufs=1)
            for c in range(NCH):
                csl = slice(c * CH, (c + 1) * CH)
                accKV = psp.tile([P, CH], dt.float32, tag="sc", bufs=4,
                                 name=f"accKV{c}")
                for k in range(NT):
                    nc.tensor.matmul(accKV, wkv[:, k, :], ht(k, c),
                                     start=(k == 0), stop=(k == NT - 1))
                accA = psp.tile([64, CH], dt.float32, tag="sc", bufs=4,
                                name=f"accA{c}")
                for k in range(NT):
                    nc.tensor.matmul(accA, w1a[:, k, :], ht(k, c),
                                     start=(k == 0), stop=(k == NT - 1))
                nc.vector.tensor_copy(kraw[:, csl], accKV[0:HD, :])
                nc.vector.tensor_copy(vt[:, csl], accKV[HD:P, :])
                # K RoPE: rotate-half via PE permutation matmul
                pmk = psp.tile([HD, CH], dt.float32, tag="od", bufs=2)
                nc.tensor.matmul(pmk, pqm[0:HD, 0:HD], kraw[:, csl],
                                 start=True, stop=True)
                t1k = phbw.tile([HD, CH], dt.bfloat16, tag="t1")
                nc.vector.tensor_mul(t1k, kraw[:, csl], cosk[:, csl])
                t2k = phbw.tile([HD, CH], dt.bfloat16, tag="t2")
                nc.vector.tensor_mul(t2k, pmk, sink[:, csl])
                nc.vector.tensor_add(kt[:, csl], t1k, t2k)
                nc.sync.dma_start(out=ktp[HD:P, csl], in_=kt[:, csl])
                # exact gelu(x) = 0.5 * x * (1 + erf(x / sqrt(2))), x = ps + b1
                pre = phbw.tile([64, CH], dt.float32, tag="pre")
                nc.vector.tensor_scalar(pre, accA, b1, None, mybir.AluOpType.add)
                er = phbw.tile([64, CH], dt.float32, tag="er")
                nc.scalar.activation(er, pre, mybir.ActivationFunctionType.Erf,
                                     bias=0.0, scale=1.0 / math.sqrt(2.0))
                nc.vector.tensor_scalar(er, er, 0.5, 0.5,
                                        mybir.AluOpType.mult, mybir.AluOpType.add)
                nc.vector.tensor_mul(hmT[:, csl], pre, er)
                # V tiles: PE transpose into unscaled vraw (the wcol gate
                # scaling runs later so the slow field chain never stalls
                # the in-order PE queue)
                for st in range(4 * c, 4 * c + 4):
                    pv = psp.tile([P, HD], dt.bfloat16, tag="od", bufs=2)
                    nc.tensor.transpose(pv, vt[:, st * P:(st + 1) * P],
                                        ident[0:HD, 0:HD])
                    nc.vector.tensor_copy(vraw[:, st, :], pv)

            # --- field row = field_scale * (hmidT^T @ W2 + b2); gate ---
            field = phb.tile([1, S], dt.float32)
            scratch = phb.tile([1, S], dt.float32)
            for c in range(NCH):
                ps = psp.tile([1, CH], dt.float32, tag="sc", bufs=4)
                nc.tensor.matmul(ps, w2, hmT[:, c * CH:(c + 1) * CH],
                                 start=True, stop=True)
                nc.vector.tensor_scalar(field[:, c * CH:(c + 1) * CH], ps,
                                        field_scale, b2_scaled,
                                        mybir.AluOpType.mult, mybir.AluOpType.add)
            ssum = phb.tile([1, 1], dt.float32)
            nc.vector.reduce_sum(ssum, field, axis=mybir.AxisListType.X)
            mean = phb.tile([1, 1], dt.float32)
            nc.vector.tensor_scalar_mul(mean, ssum, 1.0 / S)
            nc.vector.tensor_scalar(field, field, mean, None, mybir.AluOpType.subtract)
            nc.scalar.square(scratch, field)
            ss2 = phb.tile([1, 1], dt.float32)
            nc.vector.reduce_sum(ss2, scratch, axis=mybir.AxisListType.X)
            std = phb.tile([1, 1], dt.float32)
            nc.scalar.activation(std, ss2, mybir.ActivationFunctionType.Sqrt,
                                 bias=0.0, scale=1.0 / (S - 1))
            nc.vector.tensor_scalar_add(std, std, 1e-6)
            rstd = phb.tile([1, 1], dt.float32)
            nc.vector.reciprocal(rstd, std)
            gsr = phb.tile([1, 1], dt.float32)
            nc.vector.tensor_scalar_mul(gsr, rstd, gate_scale)
            # w row = exp(gate_scale * gate), into scratch
            nc.scalar.activation(scratch, field, mybir.ActivationFunctionType.Exp,
                                 bias=0.0, scale=gsr)
            # transpose the w row into per-partition columns [128, 16] via a
            # DRAM bounce (SBUF partitions are not element-addressable across
            # the partition stride, so an in-SBUF gather is illegal on HW).
            # Use the gpsimd SWDGE queue: independent of the two HWDGE rings
            # so the bounce never queues behind bulk weight traffic.
            wrow_dram = nc.dram_tensor("wrow_dram", [1, S], dt.float32)
            nc.gpsimd.dma_start(out=wrow_dram[:, :], in_=scratch)
            nc.gpsimd.dma_start(out=wcol,
                                in_=wrow_dram[0, :].rearrange("(j p) -> p j", p=P))


        # ------------- phase C setup: attention emit helpers -------------
        with tc.tile_pool(name="phc", bufs=2) as phc, \
             tc.tile_pool(name="phcs", bufs=4) as phcs, \
             tc.tile_pool(name="phd", bufs=2) as phd:

            def chunk_start(j, c):
                for ii in range(4 * c, 4 * c + 4):
                    if mb[j][ii] != 'skip':
                        return (ii % 4) * P
                return None

            pts_store = {}
            aT_store = {}

            def emit_scores(c, t):
                """Scores for head pair (2t, 2t+1): K=64 matmuls in PE
                row-groups 0 and 64 (A at base 0, B at base 64)."""
                ptsA, ptsB = {}, {}
                for j in range(NT):
                    s0 = chunk_start(j, c)
                    if s0 is None:
                        continue
                    jsl = slice(j * P, (j + 1) * P)
                    csl = slice(c * CH + s0, (c + 1) * CH)
                    psA = psp.tile([P, CH], dt.float32, tag="sc", bufs=4,
                                   name="ps_scA")
                    nc.tensor.matmul(psA[:, s0:CH], kt[:, jsl],
                                     qt_pair[t][0:HD, csl],
                                     start=True, stop=True, tile_position=(0, 0))
                    psB = psp.tile([P, CH], dt.float32, tag="sc", bufs=4,
                                   name="ps_scB")
                    nc.tensor.matmul(psB[:, s0:CH], ktp[HD:P, jsl],
                                     qt_pair[t][HD:P, csl],
                                     start=True, stop=True, tile_position=(64, 0))
                    for pts, ps, tagc in ((ptsA, psA, "pt"), (ptsB, psB, "pu")):
                        pt = phc.tile([P, CH], dt.bfloat16, tag=f"{tagc}{j}",
                                      name=f"{tagc}{j}")
                        nc.scalar.activation(pt[:, s0:CH], ps[:, s0:CH],
                                             mybir.ActivationFunctionType.Exp)
                        for ii in range(4 * c, 4 * c + 4):
                            kind = mb[j][ii]
                            if kind in ('skip', 'plain'):
                                continue
                            qq = slice((ii % 4) * P, (ii % 4 + 1) * P)
                            nc.vector.tensor_mul(pt[:, qq], pt[:, qq],
                                                 pmask[:, kind, :])
                        pts[j] = (pt, s0)
                pts_store[(c, 2 * t)] = ptsA
                pts_store[(c, 2 * t + 1)] = ptsB

            def emit_av(c, t):
                """Transposed AV for head pair t of chunk c: for each head,
                one psum [65, CH] accumulates stationary-V matmuls over key
                blocks (partial widths ride the per-element has_written
                bits).  Row 64 is the softmax denominator; its reciprocal
                row is PE-broadcast to 64 partitions and folded into the
                psum->SBUF cast, which writes attn^T for the head pair
                stacked [128, CH] - the O-projection stationary."""
                aTu = phcs.tile([P, CH], dt.bfloat16, tag="aTu", bufs=3,
                                name="aTu")
                den65 = phcs.tile([HD + 1, CH], dt.bfloat16, tag="den65",
                                  bufs=1, name="den65")
                # rows 1-63 feed the selector matmul with zero weights; they
                # must hold finite values (0 x NaN = NaN), so fill with 1.0
                nc.vector.memset(den65, 1.0)
                for hh in range(2):
                    h = 2 * t + hh
                    pts = pts_store.pop((c, h))
                    js = sorted(pts.keys())
                    pavT = psp.tile([HD + 1, CH], dt.float32, tag="av", bufs=2,
                                    name="ps_avT")
                    assert js and pts[js[0]][1] == 0, "first key block must span chunk"
                    for idx, j in enumerate(js):
                        pt, s0 = pts[j]
                        nc.tensor.matmul(pavT[:, s0:CH], vaug[:, j, :],
                                         pt[:, s0:CH],
                                         start=(idx == 0),
                                         stop=(idx == len(js) - 1))
                    # free the psum bank right away: numerators to SBUF
                    # (heads stacked), denominator row to partition hh*64
                    nc.vector.tensor_copy(aTu[hh * HD:(hh + 1) * HD, :],
                                          pavT[0:HD, :])
                    nc.vector.tensor_copy(den65[hh * HD:hh * HD + 1, :],
                                          pavT[HD:HD + 1, :])
                # one batched reciprocal covers both heads' denominator rows;
                # the broadcast + normalize run later (in this chunk's oproj
                # unit) so this slow DVE op is never on the PE critical path
                rcr65 = phcs.tile([HD + 1, CH], dt.bfloat16, tag="rcr65",
                                  bufs=3, name="rcr65")
                with nc.allow_low_precision(
                        reason="bf16 softmax-denominator reciprocal: 0.4% "
                               "per-query scale, inside the error budget"):
                    nc.vector.reciprocal(rcr65, den65)
                aT_store[(c, t)] = (aTu, rcr65)

            def emit_oproj(c, act_dc=0):
                # deferred normalize: the reciprocals were computed an entire
                # schedule unit ago, so the selector matmuls fire without
                # stalling the PE; then attn^T = unnormalized x broadcast rcp
                aTs = []
                parts = [aT_store.pop((c, t)) for t in range(HLOC // 2)]
                rcbs_t = []
                for t in range(HLOC // 2):
                    rcb = psp.tile([P, CH], dt.float32, tag="od", bufs=2,
                                   name="ps_rcb")
                    nc.tensor.matmul(rcb, sel2, parts[t][1],
                                     start=True, stop=True)
                    rcbs_t.append(rcb)
                for t in range(HLOC // 2):
                    rcbs = phcs.tile([P, CH], dt.bfloat16, tag="rcbs", bufs=2,
                                     name="rcbs")
                    nc.vector.tensor_copy(rcbs, rcbs_t[t])
                    aTt = phcs.tile([P, CH], dt.bfloat16, tag=f"aT{t}",
                                    bufs=1, name=f"aT{t}")
                    nc.vector.tensor_mul(aTt, parts[t][0], rcbs)
                    aTs.append(aTt)
                for i in range(4 * c, 4 * c + 4):
                    qsl = slice((i % 4) * P, (i % 4 + 1) * P)
                    ot = phd.tile([P, D], dt.bfloat16, tag="outsb", name="ot")
                    for dc in range(NCH):
                        po = psp.tile([P, CH], dt.float32, tag="od", bufs=2,
                                      name="ps_o")
                        dsl = slice(dc * CH, (dc + 1) * CH)
                        nc.tensor.matmul(po, aTs[0][:, qsl], wo[:, 0, dsl],
                                         start=True, stop=False)
                        nc.tensor.matmul(po, aTs[1][:, qsl], wo[:, 1, dsl],
                                         start=False, stop=True)
                        if dc < act_dc:
                            nc.scalar.copy(ot[:, dsl], po)
                        else:
                            nc.vector.tensor_copy(ot[:, dsl], po)
                    nc.sync.dma_start(out=out_d[i * P:(i + 1) * P, :], in_=ot)

            # --- pass 2: Q projections (chunk-major, hT now resident),
            # RoPE inline ---
            def emit_qchunk(t, c):
                csl = slice(c * CH, (c + 1) * CH)
                psq = psp.tile([P, CH], dt.float32, tag="sc", bufs=4,
                               name="ps_q")
                for k in range(NT):
                    nc.tensor.matmul(psq, wq[:, k, t * P:(t + 1) * P],
                                     ht(k, c),
                                     start=(k == 0), stop=(k == NT - 1))
                pair_c = phbw.tile([P, CH], dt.bfloat16, tag="pair")
                nc.vector.tensor_copy(pair_c, psq)
                pmq = psp.tile([P, CH], dt.float32, tag="od", bufs=2)
                nc.tensor.matmul(pmq, pqm, pair_c, start=True, stop=True)
                t1 = phbw.tile([P, CH], dt.bfloat16, tag="t1")
                nc.vector.tensor_mul(t1, pair_c, cos2q[:, csl])
                t2 = phbw.tile([P, CH], dt.bfloat16, tag="t2")
                nc.vector.tensor_mul(t2, pmq, sin2q[:, csl])
                nc.vector.tensor_add(qt_pair[t][:, csl], t1, t2)

            # t=0 pass with inline score blocks: exp() starts as soon as
            # the first roped q-chunk exists, ~40us earlier than emitting
            # scores after the full pass
            emit_qchunk(0, 1)
            emit_scores(1, 0)
            emit_qchunk(0, 2)
            emit_scores(2, 0)
            emit_qchunk(0, 3)
            emit_qchunk(0, 0)
            # gate scaling + denominator column; emitted after the t=0
            # ropes so the wcol DRAM bounce never stalls them
            for st in range(NT):
                nc.vector.tensor_scalar(vaug[:, st, 0:HD], vraw[:, st, :],
                                        wcol[:, st:st + 1], None,
                                        mybir.AluOpType.mult)
                nc.vector.tensor_copy(vaug[:, st, HD:HD + 1],
                                      wcol[:, st:st + 1])

            # t=1 pass interleaved with the attention schedule.  AV units
            # consume score units in emission order (pt tags have 2 bufs),
            # each chunk's oproj trails its second AV pair by >=1 unit so
            # the DVE normalize chain never stalls the in-order PE, and
            # the cheapest chunk (c=0) lands last to shrink the tail.
            emit_qchunk(1, 1)
            emit_av(1, 0)
            emit_scores(1, 1)
            emit_qchunk(1, 2)
            emit_av(2, 0)
            emit_scores(3, 0)
            emit_qchunk(1, 3)
            emit_av(1, 1)
            emit_scores(2, 1)
            emit_qchunk(1, 0)
            emit_oproj(1, act_dc=0)
            emit_av(3, 0)
            emit_scores(3, 1)
            emit_av(2, 1)
            emit_scores(0, 0)
            emit_oproj(2, act_dc=1)
            emit_av(3, 1)
            emit_scores(0, 1)
            emit_oproj(3, act_dc=2)
            emit_av(0, 0)
            emit_av(0, 1)
            emit_oproj(0, act_dc=4)

    _split_sync_waits(nc)
    return nc


def kernel(**inputs):
    global LAST_RESULT
    inp = {k: np.asarray(v) for k, v in inputs.items()}
    h = inp["hidden_states"].astype(F32).reshape(S, D)
    mask = inp["attention_mask"].astype(F32).reshape(S, S)
    cos = inp["cos"].astype(F32)
    sin = inp["sin"].astype(F32)
    Wf = inp["Wf"].astype(F32)
    W1 = inp["W1"].astype(F32)
    b1 = inp["b1"].astype(F32)
    W2 = inp["W2"].astype(F32)
    b2 = float(inp["b2"].reshape(-1)[0])
    gate_scale = float(inp["gate_scale"])
    Wq = inp["Wq"].astype(F32)
    Wk = inp["Wk"].astype(F32)
    Wv = inp["Wv"].astype(F32)
    Wo = inp["Wo"].astype(F32)

    maskT = np.ascontiguousarray(mask.T)
    mb, patterns, av_incl = _analyze_mask(maskT)
    n_pat = len(patterns)
    assert n_pat <= 64, f"too many unique mask patterns ({n_pat})"

    field_scale = float(F32(1.0 - ALPHA))
    b2_scaled = float(F32(b2) * F32(field_scale))

    nc = _build_program(mb, n_pat, av_incl, field_scale, b2_scaled, gate_scale)

    # host-side shared tensors; hT shuffled to (chunk, parity, p, kk, cols)
    hTT = np.ascontiguousarray(h.T).astype(BF16)
    hT = np.ascontiguousarray(
        hTT.reshape(NT // 2, 2, P, NCH, CH).transpose(3, 1, 2, 0, 4)
    ).reshape(NCH * 2 * P, (NT // 2) * CH)
    cosT = np.ascontiguousarray(cos.T)                       # [64, S]
    sinT = np.ascontiguousarray(sin.T)
    sin_signed = sinT.copy()
    sin_signed[0:32] = -sin_signed[0:32]
    inv_sqrt_hd = 1.0 / math.sqrt(HD)
    cos2q = np.vstack([cosT, cosT]) * inv_sqrt_hd            # [128, S]
    sin2q = np.vstack([sin_signed, sin_signed]) * inv_sqrt_hd
    cosk = cosT.astype(BF16)
    sink = sin_signed.astype(BF16)
    # rotate-half permutation (swap 32-row halves within each 64-row head)
    pq = np.zeros((P, P), dtype=BF16)
    for m in range(P):
        base = (m // HD) * HD
        r = m - base
        src = base + (r + 32) % HD
        pq[src, m] = 1.0
    w1a = (W1[:D].astype(np.float64)
           + Wf.astype(np.float64) @ W1[D:].astype(np.float64)).astype(F32).astype(BF16)

    def kmajor(w):
        # [D, F] -> [128, NT*F]: partition-major so the SBUF load is contiguous
        f = w.shape[1]
        return np.ascontiguousarray(
            w.reshape(NT, P, f).transpose(1, 0, 2).reshape(P, NT * f))

    w1a = kmajor(w1a)
    sel2_host = np.zeros((HD + 1, P), dtype=BF16)
    sel2_host[0, 0:HD] = 1.0
    sel2_host[HD, HD:P] = 1.0
    w2 = W2.reshape(64, 1).astype(BF16)
    b1c = b1.reshape(64, 1).astype(F32)
    pm = np.stack(patterns) if n_pat else None

    in_maps = []
    for c in range(NCORES):
        m = {
            "hT": hT,
            "wq": kmajor(Wq[:, c * HLOC * HD:(c + 1) * HLOC * HD].astype(BF16)),
            "wkv": kmajor(np.concatenate(
                [Wk[:, c * HD:(c + 1) * HD], Wv[:, c * HD:(c + 1) * HD]],
                axis=1).astype(BF16)),
            "wo": np.ascontiguousarray(
                Wo[c * HLOC * HD:(c + 1) * HLOC * HD, :].astype(BF16)
                .reshape(2, P, D).transpose(1, 0, 2).reshape(P, 2 * D)),
            "w1a": w1a, "w2": w2, "b1": b1c,
            "cos2q": cos2q.astype(BF16), "sin2q": sin2q.astype(BF16),
            "cosk": cosk, "sink": sink, "pq": pq, "sel2": sel2_host,
        }
        if n_pat:
            m["pmask"] = pm
        in_maps.append(m)

    trace = False
    if os.environ.get("KERNEL_TRACE"):
        try:
            import antenv.axon_hooks  # noqa: F401  (profiling shim, dev only)
            trace = True
        except ImportError:
            pass

    res = run_bass_kernel_spmd(nc, in_maps, list(range(NCORES)), trace=trace)
    LAST_RESULT = res

    out = np.zeros((S, D), dtype=F32)
    for c in range(NCORES):
        out += res.results[c]["out"].astype(F32)
    return out.reshape(1, S, D)


# revision 67
# speedup vs baseline: 1.0297x; 1.0072x over previous
"""Trainium2 Bass kernel for nn_CFHoTWrapper (sparse attention with adapter gate).

Sharding: tensor-parallel over attention heads across 8 NeuronCores.
Each core computes 4 query heads + its 1 KV head end-to-end (QKV proj,
RoPE, scores, softmax, AV, partial O-projection); the tiny adapter gate
is replicated on every core. Per-core partial outputs (bf16) are summed
on the host.

Softmax is computed without max-subtraction (scores are O(5) for these
shapes so exp() is safe in fp32), and the per-key gate bias is folded in
multiplicatively: exp(s + m + g[k]) = exp(s) * exp(m) * w[k] with
w = exp(gate_scale * gate).  w scales the V rows, and an extra all-w
column appended to V yields the softmax denominator from the same
matmul that computes the numerator.

AV is computed TRANSPOSED: stationary = augmented V block [keys, 65],
moving = exp'd score tile [keys, q-chunk].  PSUM accumulates over key
blocks with partial-width matmuls (per-element has_written handles the
causal staircase).  This kills the per-(i,j) LDWEIGHTS reloads of the
score tiles (the old AV was weight-load-bound) AND produces attn^T
directly in the [head_dim, q] layout the O-projection needs as its
stationary operand - no PE transposes, no aT copies.  The softmax
denominator lands in psum row 64; its reciprocal row is broadcast to
64 partitions with a K=1 fp32r ones-matmul and folded into the
psum->SBUF cast.

RoPE's rotate-half is a fixed row permutation, done as a PE matmul with
a permutation matrix (sign folded into the sin table) instead of
SBUF->SBUF DMA.  The adapter projection is column-tiled: even k-slices
land in psum partitions 0-63 (tile_position (0,0)), odd slices in
64-127 ((0,64)), so pairs of slices stream concurrently through the PE
and the two half-sums are merged by one DVE scalar_tensor_tensor.

DMA order keeps both HWDGE queues streaming hT from instruction 0
(small early weights first, all later-phase weights behind the hT
stream) so the PE's k-outer passes never starve and TRN2's HAM clock
gate stays at full rate.
"""

import math
import os
from contextlib import ExitStack

import numpy as np
import ml_dtypes

import concourse.bass as bass
import concourse.tile as tile
from concourse import mybir
from concourse.masks import make_identity
from concourse.bass_utils import run_bass_kernel_spmd

BF16 = ml_dtypes.bfloat16
F32 = np.float32

S = 2048
D = 2048
HD = 64
NH = 32
NKV = 8
NCORES = 8
HLOC = NH // NCORES          # 4 query heads per core
P = 128
NT = S // P                  # 16 sequence tiles of 128
NCH = 4                      # 4 sequence chunks of 512
CH = 512
ALPHA = 0.995
MASK_NEG_THRESH = -80.0      # exp() underflows to 0 below this

LAST_RESULT = None           # BassKernelResults of the last run (for test.py)


def _analyze_mask(maskT):
    """Classify [keys=128 x q=128] blocks of maskT and dedup non-trivial
    multiplicative (exp) mask patterns. maskT is [S, S] (keys, q)."""
    mb = [[None] * NT for _ in range(NT)]
    patterns = []
    pat_index = {}
    for j in range(NT):
        for i in range(NT):
            blk = maskT[j * P:(j + 1) * P, i * P:(i + 1) * P]
            if (blk < MASK_NEG_THRESH).all():
                mb[j][i] = 'skip'
            elif (blk == 0.0).all():
                mb[j][i] = 'plain'
            else:
                pat = np.exp(np.minimum(blk, 80.0)).astype(BF16)
                key = pat.tobytes()
                if key not in pat_index:
                    pat_index[key] = len(patterns)
                    patterns.append(pat)
                mb[j][i] = pat_index[key]
    av_incl = [[j for j in range(NT) if mb[j][i] != 'skip'] for i in range(NT)]
    return mb, patterns, av_incl


def _split_sync_waits(nc):
    """This walrus build supports only ONE embedded sync wait per
    instruction; hoist extra waits onto preceding sequencer NoOps."""
    for f in nc.m.functions:
        for bb in f.blocks:
            insts = bb.instructions
            idx = 0
            while idx < len(insts):
                inst = insts[idx]
                si = inst.sync_info
                if si is not None and si.on_wait and len(si.on_wait) > 1:
                    waits = list(si.on_wait)
                    for w in waits[:-1]:
                        nop = mybir.InstNoOp(
                            name=nc.get_next_instruction_name(),
                            engine=inst.engine,
                            sync_info=mybir.SyncInfo(on_wait=[w], on_update=[]),
                            bass_nofuse=True,
                        )
                        nc.register_instruction(nop)
                        insts.insert(idx, nop)
                        idx += 1
                    inst.sync_info = mybir.SyncInfo(
                        on_wait=[waits[-1]], on_update=list(si.on_update))
                idx += 1


def _build_program(mb, n_pat, av_incl, field_scale, b2_scaled, gate_scale):
    nc = bass.Bass()
    dt = mybir.dt

    # hT pre-shuffled on the host to (chunk, parity, partition, kk, cols) so
    # each (chunk, parity) load is one plain 2D DMA with 8KB-contiguous rows
    hT_d = nc.declare_dram_parameter("hT", [NCH * 2 * P, (NT // 2) * CH],
                                     dt.bfloat16, isOutput=False)
    wq_d = nc.declare_dram_parameter("wq", [P, NT * HLOC * HD], dt.bfloat16, isOutput=False)
    wkv_d = nc.declare_dram_parameter("wkv", [P, NT * 2 * HD], dt.bfloat16, isOutput=False)
    wo_d = nc.declare_dram_parameter("wo", [P, 2 * D], dt.bfloat16, isOutput=False)
    w1a_d = nc.declare_dram_parameter("w1a", [P, NT * 64], dt.bfloat16, isOutput=False)
    w2_d = nc.declare_dram_parameter("w2", [64, 1], dt.bfloat16, isOutput=False)
    b1_d = nc.declare_dram_parameter("b1", [64, 1], dt.float32, isOutput=False)
    cos2q_d = nc.declare_dram_parameter("cos2q", [P, S], dt.bfloat16, isOutput=False)
    sin2q_d = nc.declare_dram_parameter("sin2q", [P, S], dt.bfloat16, isOutput=False)
    cosk_d = nc.declare_dram_parameter("cosk", [HD, S], dt.bfloat16, isOutput=False)
    sink_d = nc.declare_dram_parameter("sink", [HD, S], dt.bfloat16, isOutput=False)
    pq_d = nc.declare_dram_parameter("pq", [P, P], dt.bfloat16, isOutput=False)
    sel2_d = nc.declare_dram_parameter("sel2", [HD + 1, P], dt.bfloat16, isOutput=False)
    if n_pat:
        pm_d = nc.declare_dram_parameter("pmask", [n_pat, P, P], dt.bfloat16, isOutput=False)
    out_d = nc.declare_dram_parameter("out", [S, D], dt.bfloat16, isOutput=True)

    with tile.TileContext(nc) as tc, ExitStack() as ctx:
        pers = ctx.enter_context(tc.tile_pool(name="pers", bufs=1))
        psp = ctx.enter_context(tc.tile_pool(name="psum", bufs=4, space="PSUM"))

        # persistent (phase-C-lifetime) tiles; DMAs are emitted inside the
        # phb block below so small early-needed weights go first on the queue
        w2 = pers.tile([64, 1], dt.bfloat16)
        b1 = pers.tile([64, 1], dt.float32)
        pqm = pers.tile([P, P], dt.bfloat16)
        # hT chunk-major: [:, c, k%2, k//2, :] with the chunk dim outermost
        # so a (chunk, parity) load lands as one contiguous 8KB run per
        # partition (128 fat descriptors per DMA, ~0.6us issue cost)
        hT3 = pers.tile([P, NCH, 2, NT // 2, CH], dt.bfloat16)

        def ht(k, c):
            return hT3[:, c, k % 2, k // 2, :]
        wq = pers.tile([P, NT, HLOC * HD], dt.bfloat16)
        if n_pat:
            pmask = pers.tile([P, n_pat, P], dt.bfloat16)
        wo = pers.tile([P, 2, D], dt.bfloat16)
        ident = pers.tile([P, P], dt.bfloat16)
        make_identity(nc, ident)
        # selector for the denominator broadcast: contraction row 0 -> ones
        # on out rows 0-63 (head A), row 64 -> ones on 64-127 (head B); all
        # other rows are zero, so one K=65 matmul fans both rows out
        sel2 = pers.tile([HD + 1, P], dt.bfloat16)


        qt_pair = [pers.tile([P, S], dt.bfloat16, tag=f"qp{t}", name=f"qp{t}")
                   for t in range(HLOC // 2)]
        ktp = pers.tile([P, S], dt.bfloat16)      # roped K at base 0 AND 64
        vaug = pers.tile([P, NT, HD + 1], dt.bfloat16)
        wcol = pers.tile([P, NT], dt.bfloat16)
        wcolf = pers.tile([P, NT], dt.float32)
        cos2q = pers.tile([P, S], dt.bfloat16)
        sin2q = pers.tile([P, S], dt.bfloat16)
        w1a = pers.tile([P, NT, 64], dt.bfloat16)

        phbw = ctx.enter_context(tc.tile_pool(name="phbw", bufs=2))
        vraw = phbw.tile([P, NT, HD], dt.bfloat16, tag="vraw", bufs=1)
        field = phbw.tile([1, S], dt.bfloat16, tag="field", bufs=1)

        def emit_qchunk(t, c):
            csl = slice(c * CH, (c + 1) * CH)
            psq = psp.tile([P, CH], dt.float32, tag="sc", bufs=4, name="ps_q")
            for k in range(NT):
                nc.tensor.matmul(psq, wq[:, k, t * P:(t + 1) * P], ht(k, c),
                                 start=(k == 0), stop=(k == NT - 1))
            pair_c = phbw.tile([P, CH], dt.bfloat16, tag="pair", bufs=1)
            nc.vector.tensor_copy(pair_c, psq)
            pmq = psp.tile([P, CH], dt.float32, tag="od", bufs=2)
            nc.tensor.matmul(pmq, pqm, pair_c, start=True, stop=True)
            t1 = phbw.tile([P, CH], dt.bfloat16, tag="t1", bufs=1)
            nc.vector.tensor_mul(t1, pair_c, cos2q[:, csl])
            t2 = phbw.tile([P, CH], dt.bfloat16, tag="t2", bufs=1)
            nc.vector.tensor_mul(t2, pmq, sin2q[:, csl])
            nc.vector.tensor_add(qt_pair[t][:, csl], t1, t2)

        def emit_adapter(c):
            """Adapter projection + gelu + W2 field slice for one chunk;
            deferred to phase C (only needed for vaug, ~70us in) so the
            KV->scores critical path never waits on it."""
            csl = slice(c * CH, (c + 1) * CH)
            accA = psp.tile([64, CH], dt.float32, tag="sc", bufs=4,
                            name=f"accA{c}")
            for k in range(NT):
                nc.tensor.matmul(accA, w1a[:, k, :], ht(k, c),
                                 start=(k == 0), stop=(k == NT - 1))
            # exact gelu(x) = 0.5 * x * (1 + erf(x / sqrt(2))), x = ps + b1
            pre = phbw.tile([64, CH], dt.bfloat16, tag="pre", bufs=1)
            nc.vector.tensor_scalar(pre, accA, b1, None, mybir.AluOpType.add)
            er = phbw.tile([64, CH], dt.bfloat16, tag="er", bufs=1)
            nc.scalar.activation(er, pre, mybir.ActivationFunctionType.Erf,
                                 bias=0.0, scale=1.0 / math.sqrt(2.0))
            nc.vector.tensor_scalar(er, er, 0.5, 0.5,
                                    mybir.AluOpType.mult, mybir.AluOpType.add)
            hmc = phbw.tile([64, CH], dt.bfloat16, tag="hmc", bufs=1)
            nc.vector.tensor_mul(hmc, pre, er)
            ps = psp.tile([1, CH], dt.float32, tag="sc", bufs=4)
            nc.tensor.matmul(ps, w2, hmc, start=True, stop=True)
            nc.vector.tensor_scalar(field[:, csl], ps,
                                    field_scale, b2_scaled,
                                    mybir.AluOpType.mult, mybir.AluOpType.add)

        def emit_field_stats():
            ssum = phbw.tile([1, 1], dt.float32, tag="fs", bufs=1)
            nc.vector.reduce_sum(ssum, field, axis=mybir.AxisListType.X)
            mean = phbw.tile([1, 1], dt.float32, tag="fm", bufs=1)
            nc.vector.tensor_scalar_mul(mean, ssum, 1.0 / S)
            nc.vector.tensor_scalar(field, field, mean, None,
                                    mybir.AluOpType.subtract)
            wrow = phbw.tile([1, S], dt.bfloat16, tag="wrow", bufs=1)
            nc.scalar.square(wrow, field)
            ss2 = phbw.tile([1, 1], dt.float32, tag="f2", bufs=1)
            nc.vector.reduce_sum(ss2, wrow, axis=mybir.AxisListType.X)
            std = phbw.tile([1, 1], dt.float32, tag="fd", bufs=1)
            nc.scalar.activation(std, ss2, mybir.ActivationFunctionType.Sqrt,
                                 bias=0.0, scale=1.0 / (S - 1))
            nc.vector.tensor_scalar_add(std, std, 1e-6)
            rstd = phbw.tile([1, 1], dt.float32, tag="fr", bufs=1)
            nc.vector.reciprocal(rstd, std)
            gsr = phbw.tile([1, 1], dt.float32, tag="fg", bufs=1)
            nc.vector.tensor_scalar_mul(gsr, rstd, gate_scale)
            # w row = exp(gate_scale * gate), overwriting the square scratch
            nc.scalar.activation(wrow, field, mybir.ActivationFunctionType.Exp,
                                 bias=0.0, scale=gsr)
            # transpose the w row into per-partition columns [128, 16] via a
            # DRAM bounce on the gpsimd SWDGE queue (independent of the two
            # HWDGE rings, and SBUF partitions aren't gather-addressable)
            wrow_dram = nc.dram_tensor("wrow_dram", [1, S], dt.bfloat16)
            nc.gpsimd.dma_start(out=wrow_dram[:, :], in_=wrow)
            nc.gpsimd.dma_start(out=wcol,
                                in_=wrow_dram[0, :].rearrange("(j p) -> p j",
                                                              p=P))

        with tc.tile_pool(name="phb", bufs=1) as phb:
            # ---- load order: tiny early weights lead, then hT arrives
            # CHUNK-major (one contiguous 2D DMA per chunk x parity, evens
            # on SP / odds on ACT) so each 512-column chunk completes early
            # and KV + K-RoPE + V-transpose + the t=0 Q chunks stream right
            # behind it; later-phase weights slotted by first-use time ----
            nc.scalar.dma_start(out=w1a, in_=w1a_d[:, :])
            wkv = phb.tile([P, NT, 2 * HD], dt.bfloat16)
            nc.scalar.dma_start(out=wkv, in_=wkv_d[:, :])
            nc.sync.dma_start(out=w2, in_=w2_d[:, :])
            nc.sync.dma_start(out=b1, in_=b1_d[:, :])
            nc.sync.dma_start(out=pqm, in_=pq_d[:, :])
            nc.sync.dma_start(out=sel2, in_=sel2_d[:, :])
            cosk = phb.tile([HD, S], dt.bfloat16)
            nc.sync.dma_start(out=cosk, in_=cosk_d[:, :])
            sink = phb.tile([HD, S], dt.bfloat16)
            nc.sync.dma_start(out=sink, in_=sink_d[:, :])

            def ht_chunk_dma(c):
                nc.sync.dma_start(out=hT3[:, c, 0, :, :],
                                  in_=hT_d[(2 * c) * P:(2 * c + 1) * P, :])
                nc.scalar.dma_start(out=hT3[:, c, 1, :, :],
                                    in_=hT_d[(2 * c + 1) * P:(2 * c + 2) * P, :])

            ht_chunk_dma(0)
            nc.scalar.dma_start(out=wq, in_=wq_d[:, :])
            nc.sync.dma_start(out=cos2q, in_=cos2q_d[:, :])
            ht_chunk_dma(1)
            nc.scalar.dma_start(out=sin2q, in_=sin2q_d[:, :])
            ht_chunk_dma(2)
            if n_pat:
                for m in range(n_pat):
                    nc.sync.dma_start(out=pmask[:, m, :], in_=pm_d[m, :, :])
            ht_chunk_dma(3)
            nc.sync.dma_start(out=wo, in_=wo_d[:, :])

            # --- per-chunk pipeline trailing the chunk-major hT stream:
            # KV projection, K-RoPE, V transposes, then (from chunk 1 on)
            # the t=0 Q chunk so the first score blocks can fire the
            # moment this pool closes ---
            for c in range(NCH):
                csl = slice(c * CH, (c + 1) * CH)
                accKV = psp.tile([P, CH], dt.float32, tag="sc", bufs=4,
                                 name=f"accKV{c}")
                for k in range(NT):
                    nc.tensor.matmul(accKV, wkv[:, k, :], ht(k, c),
                                     start=(k == 0), stop=(k == NT - 1))
                kraw = phbw.tile([HD, CH], dt.bfloat16, tag="kraw", bufs=2)
                nc.vector.tensor_copy(kraw, accKV[0:HD, :])
                vt = phbw.tile([HD, CH], dt.bfloat16, tag="vt", bufs=2)
                nc.vector.tensor_copy(vt, accKV[HD:P, :])
                # K RoPE: rotate-half via PE permutation matmul
                pmk = psp.tile([HD, CH], dt.float32, tag="od", bufs=2)
                nc.tensor.matmul(pmk, pqm[0:HD, 0:HD], kraw,
                                 start=True, stop=True)
                t1k = phbw.tile([HD, CH], dt.bfloat16, tag="t1", bufs=1)
                nc.vector.tensor_mul(t1k, kraw, cosk[:, csl])
                t2k = phbw.tile([HD, CH], dt.bfloat16, tag="t2", bufs=1)
                nc.vector.tensor_mul(t2k, pmk, sink[:, csl])
                nc.vector.tensor_add(ktp[0:HD, csl], t1k, t2k)
                nc.sync.dma_start(out=ktp[HD:P, csl], in_=ktp[0:HD, csl])
                # V tiles: PE transpose into unscaled vraw (the wcol gate
                # scaling runs later so the slow field chain never stalls
                # the in-order PE queue)
                for st in range(4 * c, 4 * c + 4):
                    pv = psp.tile([P, HD], dt.bfloat16, tag="od", bufs=2)
                    nc.tensor.transpose(pv, vt[:, (st % 4) * P:(st % 4 + 1) * P],
                                        ident[0:HD, 0:HD])
                    nc.vector.tensor_copy(vraw[:, st, :], pv)
                if c >= 1:
                    emit_qchunk(0, c)
            emit_qchunk(0, 0)


        # ------------- phase C setup: attention emit helpers -------------
        with tc.tile_pool(name="phc", bufs=2) as phc, \
             tc.tile_pool(name="phcs", bufs=4) as phcs, \
             tc.tile_pool(name="phd", bufs=2) as phd:

            def chunk_start(j, c):
                for ii in range(4 * c, 4 * c + 4):
                    if mb[j][ii] != 'skip':
                        return (ii % 4) * P
                return None

            pts_store = {}
            aT_store = {}

            def emit_scores(c, t):
                """Scores for head pair (2t, 2t+1): K=64 matmuls in PE
                row-groups 0 and 64 (A at base 0, B at base 64)."""
                ptsA, ptsB = {}, {}
                for j in range(NT):
                    s0 = chunk_start(j, c)
                    if s0 is None:
                        continue
                    jsl = slice(j * P, (j + 1) * P)
                    csl = slice(c * CH + s0, (c + 1) * CH)
                    psA = psp.tile([P, CH], dt.float32, tag="sc", bufs=4,
                                   name="ps_scA")
                    nc.tensor.matmul(psA[:, s0:CH], ktp[0:HD, jsl],
                                     qt_pair[t][0:HD, csl],
                                     start=True, stop=True, tile_position=(0, 0))
                    psB = psp.tile([P, CH], dt.float32, tag="sc", bufs=4,
                                   name="ps_scB")
                    nc.tensor.matmul(psB[:, s0:CH], ktp[HD:P, jsl],
                                     qt_pair[t][HD:P, csl],
                                     start=True, stop=True, tile_position=(64, 0))
                    for pts, ps, tagc in ((ptsA, psA, "pt"), (ptsB, psB, "pu")):
                        pt = phc.tile([P, CH], dt.bfloat16, tag=f"{tagc}{j}",
                                      name=f"{tagc}{j}")
                        nc.scalar.activation(pt[:, s0:CH], ps[:, s0:CH],
                                             mybir.ActivationFunctionType.Exp)
                        for ii in range(4 * c, 4 * c + 4):
                            kind = mb[j][ii]
                            if kind in ('skip', 'plain'):
                                continue
                            qq = slice((ii % 4) * P, (ii % 4 + 1) * P)
                            nc.vector.tensor_mul(pt[:, qq], pt[:, qq],
                                                 pmask[:, kind, :])
                        pts[j] = (pt, s0)
                pts_store[(c, 2 * t)] = ptsA
                pts_store[(c, 2 * t + 1)] = ptsB

            def emit_av(c, t):
                """Transposed AV for head pair t of chunk c: for each head,
                one psum [65, CH] accumulates stationary-V matmuls over key
                blocks (partial widths ride the per-element has_written
                bits).  Row 64 is the softmax denominator; its reciprocal
                row is PE-broadcast to 64 partitions and folded into the
                psum->SBUF cast, which writes attn^T for the head pair
                stacked [128, CH] - the O-projection stationary."""
                aTu = phcs.tile([P, CH], dt.bfloat16, tag="aTu", bufs=3,
                                name="aTu")
                den65 = phcs.tile([HD + 1, CH], dt.bfloat16, tag="den65",
                                  bufs=1, name="den65")
                # rows 1-63 feed the selector matmul with zero weights; they
                # must hold finite values (0 x NaN = NaN), so fill with 1.0
                nc.vector.memset(den65, 1.0)
                for hh in range(2):
                    h = 2 * t + hh
                    pts = pts_store.pop((c, h))
                    js = sorted(pts.keys())
                    pavT = psp.tile([HD + 1, CH], dt.float32, tag="av", bufs=2,
                                    name="ps_avT")
                    assert js and pts[js[0]][1] == 0, "first key block must span chunk"
                    for idx, j in enumerate(js):
                        pt, s0 = pts[j]
                        nc.tensor.matmul(pavT[:, s0:CH], vaug[:, j, :],
                                         pt[:, s0:CH],
                                         start=(idx == 0),
                                         stop=(idx == len(js) - 1))
                    # free the psum bank right away: numerators to SBUF
                    # (heads stacked), denominator row to partition hh*64
                    nc.vector.tensor_copy(aTu[hh * HD:(hh + 1) * HD, :],
                                          pavT[0:HD, :])
                    nc.vector.tensor_copy(den65[hh * HD:hh * HD + 1, :],
                                          pavT[HD:HD + 1, :])
                # one batched reciprocal covers both heads' denominator rows;
                # the broadcast + normalize run later (in this chunk's oproj
                # unit) so this slow DVE op is never on the PE critical path
                rcr65 = phcs.tile([HD + 1, CH], dt.bfloat16, tag="rcr65",
                                  bufs=3, name="rcr65")
                with nc.allow_low_precision(
                        reason="bf16 softmax-denominator reciprocal: 0.4% "
                               "per-query scale, inside the error budget"):
                    nc.vector.reciprocal(rcr65, den65)
                aT_store[(c, t)] = (aTu, rcr65)

            def emit_oproj(c, act_dc=0):
                # deferred normalize: the reciprocals were computed an entire
                # schedule unit ago, so the selector matmuls fire without
                # stalling the PE; then attn^T = unnormalized x broadcast rcp
                aTs = []
                parts = [aT_store.pop((c, t)) for t in range(HLOC // 2)]
                rcbs_t = []
                for t in range(HLOC // 2):
                    rcb = psp.tile([P, CH], dt.float32, tag="od", bufs=2,
                                   name="ps_rcb")
                    nc.tensor.matmul(rcb, sel2, parts[t][1],
                                     start=True, stop=True)
                    rcbs_t.append(rcb)
                for t in range(HLOC // 2):
                    rcbs = phcs.tile([P, CH], dt.bfloat16, tag="rcbs", bufs=1,
                                     name="rcbs")
                    nc.vector.tensor_copy(rcbs, rcbs_t[t])
                    aTt = phcs.tile([P, CH], dt.bfloat16, tag=f"aT{t}",
                                    bufs=1, name=f"aT{t}")
                    nc.vector.tensor_mul(aTt, parts[t][0], rcbs)
                    aTs.append(aTt)
                for i in range(4 * c, 4 * c + 4):
                    qsl = slice((i % 4) * P, (i % 4 + 1) * P)
                    ot = phd.tile([P, D], dt.bfloat16, tag="outsb", name="ot")
                    for dc in range(NCH):
                        po = psp.tile([P, CH], dt.float32, tag="od", bufs=2,
                                      name="ps_o")
                        dsl = slice(dc * CH, (dc + 1) * CH)
                        nc.tensor.matmul(po, aTs[0][:, qsl], wo[:, 0, dsl],
                                         start=True, stop=False)
                        nc.tensor.matmul(po, aTs[1][:, qsl], wo[:, 1, dsl],
                                         start=False, stop=True)
                        if dc < act_dc:
                            nc.scalar.copy(ot[:, dsl], po)
                        else:
                            nc.vector.tensor_copy(ot[:, dsl], po)
                    nc.sync.dma_start(out=out_d[i * P:(i + 1) * P, :], in_=ot)

            # The t=0 Q chunks were already emitted inside the phase-B
            # chunk loop, so the first score blocks fire immediately; the
            # deferred adapter chunks slot between the early score units
            # (exp keeps ACT busy while the PE grinds the adapter), then
            # the field stats -> wcol bounce -> vaug, then the t=1 pass
            # interleaved with the attention schedule.  AV units consume
            # score units in emission order (pt tags have 2 bufs), each
            # chunk's oproj trails its second AV pair so the DVE normalize
            # chain never stalls the in-order PE, and the cheapest chunk
            # (c=0) lands last to shrink the tail.
            emit_scores(1, 0)
            emit_adapter(0)
            emit_scores(2, 0)
            emit_adapter(1)
            emit_adapter(2)
            emit_adapter(3)
            emit_field_stats()
            emit_qchunk(1, 1)
            # gate scaling + denominator column (after the wcol bounce);
            # per-partition scalars must be f32 for DVE mult
            nc.vector.tensor_copy(wcolf, wcol)
            for st in range(NT):
                nc.vector.tensor_scalar(vaug[:, st, 0:HD], vraw[:, st, :],
                                        wcolf[:, st:st + 1], None,
                                        mybir.AluOpType.mult)
                nc.vector.tensor_copy(vaug[:, st, HD:HD + 1],
                                      wcolf[:, st:st + 1])
            emit_av(1, 0)
            emit_scores(3, 0)
            emit_qchunk(1, 2)
            emit_av(2, 0)
            emit_scores(1, 1)
            emit_qchunk(1, 3)
            emit_av(1, 1)
            emit_scores(2, 1)
            emit_qchunk(1, 0)
            emit_oproj(1, act_dc=0)
            emit_av(3, 0)
            emit_scores(3, 1)
            emit_av(2, 1)
            emit_scores(0, 0)
            emit_oproj(2, act_dc=1)
            emit_av(3, 1)
            emit_scores(0, 1)
            emit_oproj(3, act_dc=2)
            emit_av(0, 0)
            emit_av(0, 1)
            emit_oproj(0, act_dc=4)

    _split_sync_waits(nc)
    return nc


def kernel(**inputs):
    global LAST_RESULT
    inp = {k: np.asarray(v) for k, v in inputs.items()}
    h = inp["hidden_states"].astype(F32).reshape(S, D)
    mask = inp["attention_mask"].astype(F32).reshape(S, S)
    cos = inp["cos"].astype(F32)
    sin = inp["sin"].astype(F32)
    Wf = inp["Wf"].astype(F32)
    W1 = inp["W1"].astype(F32)
    b1 = inp["b1"].astype(F32)
    W2 = inp["W2"].astype(F32)
    b2 = float(inp["b2"].reshape(-1)[0])
    gate_scale = float(inp["gate_scale"])
    Wq = inp["Wq"].astype(F32)
    Wk = inp["Wk"].astype(F32)
    Wv = inp["Wv"].astype(F32)
    Wo = inp["Wo"].astype(F32)

    maskT = np.ascontiguousarray(mask.T)
    mb, patterns, av_incl = _analyze_mask(maskT)
    n_pat = len(patterns)
    assert n_pat <= 64, f"too many unique mask patterns ({n_pat})"

    field_scale = float(F32(1.0 - ALPHA))
    b2_scaled = float(F32(b2) * F32(field_scale))

    nc = _build_program(mb, n_pat, av_incl, field_scale, b2_scaled, gate_scale)

    # host-side shared tensors; hT shuffled to (chunk, parity, p, kk, cols)
    hTT = np.ascontiguousarray(h.T).astype(BF16)
    hT = np.ascontiguousarray(
        hTT.reshape(NT // 2, 2, P, NCH, CH).transpose(3, 1, 2, 0, 4)
    ).reshape(NCH * 2 * P, (NT // 2) * CH)
    cosT = np.ascontiguousarray(cos.T)                       # [64, S]
    sinT = np.ascontiguousarray(sin.T)
    sin_signed = sinT.copy()
    sin_signed[0:32] = -sin_signed[0:32]
    inv_sqrt_hd = 1.0 / math.sqrt(HD)
    cos2q = np.vstack([cosT, cosT]) * inv_sqrt_hd            # [128, S]
    sin2q = np.vstack([sin_signed, sin_signed]) * inv_sqrt_hd
    cosk = cosT.astype(BF16)
    sink = sin_signed.astype(BF16)
    # rotate-half permutation (swap 32-row halves within each 64-row head)
    pq = np.zeros((P, P), dtype=BF16)
    for m in range(P):
        base = (m // HD) * HD
        r = m - base
        src = base + (r + 32) % HD
        pq[src, m] = 1.0
    w1a = (W1[:D].astype(np.float64)
           + Wf.astype(np.float64) @ W1[D:].astype(np.float64)).astype(F32).astype(BF16)

    def kmajor(w):
        # [D, F] -> [128, NT*F]: partition-major so the SBUF load is contiguous
        f = w.shape[1]
        return np.ascontiguousarray(
            w.reshape(NT, P, f).transpose(1, 0, 2).reshape(P, NT * f))

    w1a = kmajor(w1a)
    sel2_host = np.zeros((HD + 1, P), dtype=BF16)
    sel2_host[0, 0:HD] = 1.0
    sel2_host[HD, HD:P] = 1.0
    w2 = W2.reshape(64, 1).astype(BF16)
    b1c = b1.reshape(64, 1).astype(F32)
    pm = np.stack(patterns) if n_pat else None

    in_maps = []
    for c in range(NCORES):
        m = {
            "hT": hT,
            "wq": kmajor(Wq[:, c * HLOC * HD:(c + 1) * HLOC * HD].astype(BF16)),
            "wkv": kmajor(np.concatenate(
                [Wk[:, c * HD:(c + 1) * HD], Wv[:, c * HD:(c + 1) * HD]],
                axis=1).astype(BF16)),
            "wo": np.ascontiguousarray(
                Wo[c * HLOC * HD:(c + 1) * HLOC * HD, :].astype(BF16)
                .reshape(2, P, D).transpose(1, 0, 2).reshape(P, 2 * D)),
            "w1a": w1a, "w2": w2, "b1": b1c,
            "cos2q": cos2q.astype(BF16), "sin2q": sin2q.astype(BF16),
            "cosk": cosk, "sink": sink, "pq": pq, "sel2": sel2_host,
        }
        if n_pat:
            m["pmask"] = pm
        in_maps.append(m)

    trace = False
    if os.environ.get("KERNEL_TRACE"):
        try:
            import antenv.axon_hooks  # noqa: F401  (profiling shim, dev only)
            trace = True
        except ImportError:
            pass

    res = run_bass_kernel_spmd(nc, in_maps, list(range(NCORES)), trace=trace)
    LAST_RESULT = res

    out = np.zeros((S, D), dtype=F32)
    for c in range(NCORES):
        out += res.results[c]["out"].astype(F32)
    return out.reshape(1, S, D)
